# revision 32
# baseline (speedup 1.0000x reference)
"""Bass/Tile kernel for nn_DeepseekV3MLPMoEModel on 8 trn2 cores.

Sharding: data-parallel over tokens (T/8 per core) for attention/MLP/lm_head
(vocab-sharded), expert-parallel for the MoE (1 expert/core, dense over all
tokens, ReduceScatter of the weighted sum).

Residual stream layout on device: xT [D(part-chunks of 128), T_loc] (f32r).
"""
import sys
sys.path.insert(0, "/opt/trn_rl_repo")
import numpy as np
import concourse.bass as bass
import concourse.mybir as mybir
import concourse.tile as tile
from concourse import bacc
from concourse.bass_utils import run_bass_kernel_spmd
from concourse.masks import make_identity

F32 = mybir.dt.float32
BF16 = mybir.dt.bfloat16
F32R = mybir.dt.float32r
I32 = mybir.dt.int32
AF = mybir.ActivationFunctionType
OP = mybir.AluOpType
AX = mybir.AxisListType

FULL_CFG = dict(B=2, S=2048, D=1024, H=16, F=2048, E=8, V=32000, L=2, NC=8, G=4,
                C2=192)
MINI_CFG = dict(B=2, S=512, D=256, H=4, F=512, E=8, V=1024, L=2, NC=8, G=4,
                C2=64)


def derived(cfg):
    c = dict(cfg)
    c["T"] = c["B"] * c["S"]
    c["TC"] = c["T"] // c["NC"]          # tokens per core
    c["TCH"] = c["TC"] // 128            # token tiles per core
    c["DK"] = c["D"] // 128              # D chunks
    c["FK"] = c["F"] // 128              # F chunks
    c["VC"] = c["V"] // c["NC"]          # vocab per core
    c["VCP"] = ((c["VC"] + 127) // 128) * 128
    c["VCK"] = c["VCP"] // 128
    c["VS"] = c["D"] // c["TC"]          # v slots per token-tile in kv pack
    c["SLOTS"] = c["DK"] + c["TCH"] * c["VS"]
    c["dh"] = c["D"] // c["H"]
    assert c["dh"] == 64
    return c


# ---------------------------------------------------------------- host prep

def lhsT_tiles(W, bf16=True):
    """W [M, K] (for out = x @ W.T) -> [M/128, 128(ki), K/128(ko), 128(mm)]."""
    import ml_dtypes
    M, K = W.shape
    Wt = np.ascontiguousarray(W.T)
    r = np.ascontiguousarray(
        Wt.reshape(K // 128, 128, M // 128, 128).transpose(2, 1, 0, 3))
    return r.astype(ml_dtypes.bfloat16) if bf16 else r


def rhs_tiles(W, bf16=False):
    """W [N, K] (used as rhs [K, N]) -> [K/128, 128, N]."""
    import ml_dtypes
    N, K = W.shape
    r = np.ascontiguousarray(W.T.reshape(K // 128, 128, N))
    return r.astype(ml_dtypes.bfloat16) if bf16 else r


def pp_cols(b):
    """b [M] -> [128, M/128]: column m holds b[m*128:(m+1)*128]."""
    return np.ascontiguousarray(b.reshape(-1, 128).T)


def prep_in_maps(inputs, cfg):
    c = derived(cfg)
    NC, L, D, E = c["NC"], c["L"], c["D"], c["E"]
    VC, VCP = c["VC"], c["VCP"]
    f32 = np.float32

    tokens = np.asarray(inputs["tokens"]).astype(np.int64).reshape(-1)  # [T]
    emb = np.asarray(inputs["emb"], f32)

    shared = {}
    for l in range(L):
        ipw = np.asarray(inputs["in_proj_w"][l], f32)     # [3D, D]
        ipb = np.asarray(inputs["in_proj_b"][l], f32)     # [3D]
        bqk = ipb[:2 * D].copy()
        bqk[:D] *= 0.125
        shared[f"wqk{l}"] = lhsT_tiles(ipw[:2 * D], bf16=True)
        shared[f"bqk{l}"] = pp_cols(bqk)
        shared[f"wv{l}"] = rhs_tiles(ipw[2 * D:], bf16=True)
        shared[f"bv{l}"] = ipb[2 * D:].reshape(1, D).copy()
        shared[f"wo{l}"] = lhsT_tiles(np.asarray(inputs["out_proj_w"][l], f32), bf16=True)
        shared[f"bo{l}"] = pp_cols(np.asarray(inputs["out_proj_b"][l], f32))
        for nm in ("ln1_w", "ln1_b", "ln2_w", "ln2_b"):
            shared[f"{nm.replace('_','')}{l}"] = pp_cols(np.asarray(inputs[nm][l], f32))
        shared[f"wg{l}"] = lhsT_tiles(np.asarray(inputs["ds_gate_w"][l], f32), bf16=True)
        shared[f"wu{l}"] = lhsT_tiles(np.asarray(inputs["ds_up_w"][l], f32), bf16=True)
        shared[f"wd{l}"] = lhsT_tiles(np.asarray(inputs["ds_down_w"][l], f32), bf16=True)
        shared[f"gw{l}"] = rhs_tiles(np.asarray(inputs["gate_w"][l], f32))
        shared[f"gb{l}"] = np.asarray(inputs["gate_b"][l], f32).reshape(1, E).copy()
    shared["rmsw"] = pp_cols(np.asarray(inputs["rms_w"], f32))
    shared["ones_mat"] = np.ones((128, 128), f32)
    import ml_dtypes
    shared["ones_bf"] = np.ones((128, 64), ml_dtypes.bfloat16)
    shared["triu"] = np.triu(np.ones((128, 128), f32), 1)
    shared["ebase"] = (np.arange(E) * 64).astype(f32).reshape(1, E)

    in_maps = []
    for core in range(NC):
        m = dict(shared)
        lo = core * VC
        m["embrows"] = emb  # replicated full table
        loc = tokens[core * (len(tokens) // NC):(core + 1) * (len(tokens) // NC)]
        m["tokidx"] = np.ascontiguousarray(
            loc.reshape(-1, 128).T.astype(np.int32))  # [128, TC/128]
        esl = np.zeros((VCP, D), f32)
        esl[:VC] = emb[lo:lo + VC]
        m["embT"] = lhsT_tiles(esl, bf16=c.get("lm_bf16", True))
        for l in range(L):
            m[f"w1{l}"] = lhsT_tiles(np.asarray(inputs["moe_w1"][l, core], f32), bf16=True)
            m[f"b1{l}"] = pp_cols(np.asarray(inputs["moe_b1"][l, core], f32))
            m[f"w2{l}"] = lhsT_tiles(np.asarray(inputs["moe_w2"][l, core], f32), bf16=True)
            m[f"b2{l}"] = pp_cols(np.asarray(inputs["moe_b2"][l, core], f32))
        in_maps.append(m)
    return in_maps


def assemble_logits(results, cfg):
    c = derived(cfg)
    B, S, V, VC = c["B"], c["S"], c["V"], c["VC"]
    out = np.empty((B, S, V), np.float32)
    for core, r in enumerate(results):
        lg = r["logits"]  # [VC, T]
        out[:, :, core * VC:(core + 1) * VC] = lg.T.reshape(B, S, VC)
    return out


# ---------------------------------------------------------------- device code

def build_nc(cfg):
    c = derived(cfg)
    L, D, E = c["L"], c["D"], c["E"]
    DK, FK = c["DK"], c["FK"]
    VC, VCK = c["VC"], c["VCK"]
    T = c["T"]

    nc = bacc.Bacc(None)
    P = {}

    def par(name, shape, dt):
        P[name] = nc.dram_tensor(name, shape, dt, kind="ExternalInput")

    par("tokidx", [128, T // (8 * 128)], I32)
    par("ones_mat", [128, 128], F32R)
    par("ones_bf", [128, 64], BF16)
    par("triu", [128, 128], F32R)
    par("ebase", [1, E], F32R)
    par("embrows", [c["V"], D], F32)
    par("embT", [VCK, 128, DK, 128], BF16 if c.get("lm_bf16", True) else F32R)
    for l in range(L):
        par(f"wqk{l}", [2 * DK, 128, DK, 128], BF16)
        par(f"bqk{l}", [128, 2 * DK], F32)
        par(f"wv{l}", [DK, 128, D], BF16)
        par(f"bv{l}", [1, D], F32R)
        par(f"wo{l}", [DK, 128, DK, 128], BF16)
        par(f"bo{l}", [128, DK], F32)
        for nm in ("ln1w", "ln1b", "ln2w", "ln2b"):
            par(f"{nm}{l}", [128, DK], F32)
        par(f"wg{l}", [FK, 128, DK, 128], BF16)
        par(f"wu{l}", [FK, 128, DK, 128], BF16)
        par(f"wd{l}", [DK, 128, FK, 128], BF16)
        par(f"gw{l}", [DK, 128, E], F32)
        par(f"gb{l}", [1, E], F32R)
        par(f"w1{l}", [FK, 128, DK, 128], BF16)
        par(f"b1{l}", [128, FK], F32)
        par(f"w2{l}", [DK, 128, FK, 128], BF16)
        par(f"b2{l}", [128, DK], F32)
    par("rmsw", [128, DK], F32)
    OUT = nc.dram_tensor("logits", [VC, T], F32, kind="ExternalOutput")

    with tile.TileContext(nc) as tc:
        _emit(nc, tc, P, OUT, c)
    nc.compile()
    return nc


def _emit(nc, tc, P, OUT, c):
    NC, L, D, H, F, E = c["NC"], c["L"], c["D"], c["H"], c["F"], c["E"]
    TC, TCH, DK, FK = c["TC"], c["TCH"], c["DK"], c["FK"]
    VC, VCK, VS, SLOTS = c["VC"], c["VCK"], c["VS"], c["SLOTS"]
    G, T = c["G"], c["T"]
    KCH = G * TCH
    TK = T // 128
    NDN = max(1, D // 512)
    NW = min(512, D)
    GRP_KV = [list(range(g * G, (g + 1) * G)) for g in range(NC // G)]
    GRP_ALL = [list(range(NC))]

    from contextlib import ExitStack
    es = ExitStack()
    cst = es.enter_context(tc.tile_pool(name="cst", bufs=1))
    sbt = es.enter_context(tc.tile_pool(name="sbt", bufs=2))
    lnp = es.enter_context(tc.tile_pool(name="lnp", bufs=2))
    xlp = es.enter_context(tc.tile_pool(name="xlp", bufs=1))
    psm = es.enter_context(tc.tile_pool(name="psm", bufs=3, space="PSUM"))
    pst = es.enter_context(tc.tile_pool(name="pst", bufs=2, space="PSUM"))
    ptr = es.enter_context(tc.tile_pool(name="ptr", bufs=1, space="PSUM"))
    drp = es.enter_context(tc.tile_pool(name="drp", bufs=1, space="DRAM"))

    dbg_on = c.get("debug", False)

    def dbg(name, ap):
        if not dbg_on:
            return
        t = nc.dram_tensor(f"dbg_{name}", list(ap.shape), ap.dtype,
                           kind="ExternalOutput")
        nc.sync.dma_start(t[:], ap)

    ident = cst.tile([128, 128], F32, name="ident")
    make_identity(nc, ident)
    identB = cst.tile([128, 128], BF16, name="identB")
    nc.vector.tensor_copy(identB[:], ident[:])
    ones_m = cst.tile([128, 128], F32R, name="ones_m")
    nc.sync.dma_start(ones_m[:], P["ones_mat"][:])
    triu_sb = cst.tile([128, 128], F32R, name="triu_sb")
    nc.sync.dma_start(triu_sb[:], P["triu"][:])
    ebase_sb = cst.tile([1, E], F32R, name="ebase_sb")
    nc.sync.dma_start(ebase_sb[:], P["ebase"][:])
    ptb = es.enter_context(tc.tile_pool(name="ptb", bufs=2, space="PSUM"))
    eps5 = cst.tile([128, 1], F32, name="eps5")
    nc.gpsimd.memset(eps5[:], 1e-5)
    eps6 = cst.tile([128, 1], F32, name="eps6")
    nc.gpsimd.memset(eps6[:], 1e-6)
    xT = cst.tile([128, DK, TC], F32R, name="xT")
    xB = cst.tile([128, DK, TC], BF16, name="xB")
    
    KCH_ = G * TCH


    # ---------------- embedding: gather own tokens from replicated table
    with tc.tile_pool(name="emb_ph", bufs=3) as ph:
        idx_sb = ph.tile([128, TCH], I32, name="idx_sb", bufs=1)
        nc.sync.dma_start(idx_sb[:], P["tokidx"][:])
        sqrt_d = float(np.sqrt(c["D"]))
        for tm in range(TCH):
            ge = ph.tile([128, D], F32, tag="ge")
            nc.gpsimd.indirect_dma_start(
                out=ge[:], out_offset=None, in_=P["embrows"][:],
                in_offset=bass.IndirectOffsetOnAxis(ap=idx_sb[:, tm:tm + 1], axis=0))
            for k in range(DK):
                pt = ptr.tile([128, 128], F32, tag="ptr")
                nc.tensor.transpose(pt[:], ge[:, k * 128:(k + 1) * 128], ident[:])
                nc.scalar.activation(xT[:, k, tm * 128:(tm + 1) * 128], pt[:],
                                     AF.Copy, scale=sqrt_d)
                nc.vector.tensor_copy(xB[:, k, tm * 128:(tm + 1) * 128],
                                      xT[:, k, tm * 128:(tm + 1) * 128])
    dbg("x0T", xT[:])

    # ---------------- LN helper (matmul stats, replicated across partitions)
    def layer_norm_(dst, src, wcols, bcols, eps, skip_mean=False, bdst=None):
        eps = eps5[:, 0:1] if eps == 1e-5 else eps6[:, 0:1]
        ps1 = None if skip_mean else pst.tile([128, TC], F32, tag="pstat")
        ps2 = pst.tile([128, TC], F32, tag="pstat")
        for k in range(DK):
            sq = lnp.tile([128, TC], F32R, tag="sq")
            nc.vector.tensor_tensor(sq[:], src[:, k, :], src[:, k, :], OP.mult)
            if not skip_mean:
                nc.tensor.matmul(ps1[:], ones_m[:], src[:, k, :],
                                 start=(k == 0), stop=(k == DK - 1))
            nc.tensor.matmul(ps2[:], ones_m[:], sq[:],
                             start=(k == 0), stop=(k == DK - 1))
        e2 = lnp.tile([128, TC], F32, tag="stmp")
        nc.scalar.activation(e2[:], ps2[:], AF.Copy, scale=1.0 / c["D"])
        if not skip_mean:
            mu = lnp.tile([128, TC], F32, tag="smu", bufs=1)
            nc.scalar.activation(mu[:], ps1[:], AF.Copy, scale=1.0 / c["D"])
            var = lnp.tile([128, TC], F32, tag="stmp")
            nc.vector.tensor_tensor(var[:], mu[:], mu[:], OP.mult)
            nc.vector.tensor_tensor(var[:], e2[:], var[:], OP.subtract)
        else:
            var = e2
        sd = lnp.tile([128, TC], F32, tag="stmp")
        nc.scalar.activation(sd[:], var[:], AF.Sqrt, bias=eps)
        rstd = lnp.tile([128, TC], F32, tag="srstd", bufs=1)
        nc.vector.reciprocal(rstd[:], sd[:])
        for k in range(DK):
            t1 = lnp.tile([128, TC], F32, tag="lnt")
            if not skip_mean:
                nc.vector.tensor_tensor(t1[:], src[:, k, :], mu[:], OP.subtract)
                nc.vector.tensor_tensor(t1[:], t1[:], rstd[:], OP.mult)
            else:
                nc.vector.tensor_tensor(t1[:], src[:, k, :], rstd[:], OP.mult)
            if bcols is not None:
                nc.vector.tensor_scalar(dst[:, k, :], t1[:],
                                        wcols[:, k:k + 1], bcols[:, k:k + 1],
                                        OP.mult, OP.add)
            else:
                nc.vector.tensor_scalar_mul(dst[:, k, :], t1[:], wcols[:, k:k + 1])
            if bdst is not None:
                nc.scalar.activation(bdst[:, k, :], dst[:, k, :], AF.Copy)

    # ---------------- layers
    for l in range(L):
        lb = {}
        for nm in ("bqk", "bo", "ln1w", "ln1b", "ln2w", "ln2b", "b1", "b2"):
            w = P[f"{nm}{l}"].shape[1]
            t = cst.tile([128, w], F32, name=f"{nm}{l}_sb", tag=f"c_{nm}")
            nc.sync.dma_start(t[:], P[f"{nm}{l}"][:])
            lb[nm] = t
        bv1 = cst.tile([1, D], F32R, name=f"bv1_{l}", tag="c_bv1")
        nc.sync.dma_start(bv1[:], P[f"bv{l}"][:])
        bv = cst.tile([128, D], F32, name=f"bv{l}_sb", tag="c_bv")
        for dn in range(NDN):
            psb = psm.tile([128, NW], F32, tag="psmm")
            nc.tensor.matmul(psb[:], ones_m[0:1, :],
                             bv1[0:1, dn * NW:(dn + 1) * NW], start=True, stop=True)
            nc.vector.tensor_copy(bv[:, dn * NW:(dn + 1) * NW], psb[:])
        gb1 = cst.tile([1, E], F32R, name=f"gb1_{l}", tag="c_gb1")
        nc.sync.dma_start(gb1[:], P[f"gb{l}"][:])
        psgb = psm.tile([128, E], F32, tag="psmm")
        nc.tensor.matmul(psgb[:], ones_m[0:1, :], gb1[0:1, :], start=True, stop=True)
        gb = cst.tile([128, E], F32, name=f"gb{l}_sb", tag="c_gb")
        nc.vector.tensor_copy(gb[:], psgb[:])

        assert NDN == 2 and NW == TC and VS == 2
        kv_ink = drp.tile([DK, 128, TC], BF16, name="kvink", tag="kvink")
        kv_allk = drp.tile([G, DK, 128, TC], BF16, name="kvallk", tag="kvallk")
        kv_inv = [drp.tile([TCH, 128, TC], BF16, name=f"kvinv{dn}",
                           tag=f"kvinv{dn}") for dn in range(NDN)]
        kv_allv = [drp.tile([G, TCH, 128, TC], BF16, name=f"kvallv{dn}",
                            tag=f"kvallv{dn}") for dn in range(NDN)]

        # --- qkv phase: k first (gather overlaps q), then q, then v
        with tc.tile_pool(name="qp", bufs=1) as qp:
            q_sb = qp.tile([128, DK, TC], BF16, tag="q_sb")
            with (
                tc.tile_pool(name="qphw", bufs=4) as qphw,
                tc.tile_pool(name="qphk", bufs=2) as qphk,
                tc.tile_pool(name="qpv", bufs=1) as qpv,
            ):
                for m in range(2 * DK):
                    mm = (m + DK) % (2 * DK)        # k chunks first
                    wt = qphw.tile([128, DK, 128], BF16, tag="wt")
                    nc.sync.dma_start(wt[:], P[f"wqk{l}"][mm])
                    ps = psm.tile([128, TC], F32, tag="psmm")
                    for k in range(DK):
                        nc.tensor.matmul(ps[:], wt[:, k, :], xB[:, k, :],
                                         start=(k == 0), stop=(k == DK - 1))
                    if mm < DK:
                        nc.scalar.activation(q_sb[:, mm, :], ps[:], AF.Identity,
                                             scale=0.125, bias=lb["bqk"][:, mm:mm + 1])
                    else:
                        kt = qphk.tile([128, TC], BF16, tag="kt")
                        nc.scalar.activation(kt[:], ps[:], AF.Identity,
                                             bias=lb["bqk"][:, mm:mm + 1])
                        nc.sync.dma_start(kv_ink[mm - DK], kt[:])
                        if mm == 2 * DK - 1:
                            nc.gpsimd.collective_compute(
                                "AllGather", OP.bypass, replica_groups=GRP_KV,
                                ins=[kv_ink[:]], outs=[kv_allk[:]])
                for dn in range(NDN):
                    wv = qpv.tile([128, DK, NW], BF16, tag="wv")
                    for k in range(DK):
                        nc.sync.dma_start(wv[:, k, :],
                                          P[f"wv{l}"][k, :, dn * NW:(dn + 1) * NW])
                    for tm in range(TCH):
                        ps = psm.tile([128, NW], F32, tag="psmm")
                        for k in range(DK):
                            nc.tensor.matmul(ps[:], xB[:, k, tm * 128:(tm + 1) * 128],
                                             wv[:, k, :],
                                             start=(k == 0), stop=(k == DK - 1))
                        vt = qphk.tile([128, NW], BF16, tag="vt")
                        nc.vector.tensor_tensor(
                            vt[:], ps[:], bv[:, dn * NW:(dn + 1) * NW], OP.add)
                        nc.sync.dma_start(kv_inv[dn][tm], vt[:])
                    nc.gpsimd.collective_compute(
                        "AllGather", OP.bypass, replica_groups=GRP_KV,
                        ins=[kv_inv[dn][:]], outs=[kv_allv[dn][:]])
            if l == 0:
                dbg("q0", q_sb[:])

            # --- attention (q_sb in scope)
            with tc.tile_pool(name="aoT", bufs=1) as aoTp:
                oT = aoTp.tile([128, DK, TC], BF16, tag="oT")
                vh2 = aoTp.tile([128, 2, KCH, 128], BF16, tag="vh2")
                for b_ in range(2):
                    for kc_ in range(KCH):
                        nc.sync.dma_start(vh2[:, b_, kc_, 64:128],
                                          P["ones_bf"][:, 0:64])
                with (
                    tc.tile_pool(name="aph", bufs=2) as aph,
                    tc.tile_pool(name="apT", bufs=1) as apTp,
                ):
                    for h in range(H):
                        qm, qoff = h // 2, 64 * (h % 2)
                        kh = aph.tile([128, G, TC], BF16, tag="kh")
                        for g in range(G):
                            nc.sync.dma_start(kh[qoff:qoff + 64, g, :],
                                              kv_allk[g, qm, qoff:qoff + 64, :])
                        s_v, off_v = h // (TC // 64), (64 * h) % TC
                        for g in range(G):
                            for tm in range(TCH):
                                nc.sync.dma_start(
                                    vh2[:, h % 2, g * TCH + tm, 0:64],
                                    kv_allv[s_v][g, tm, :, off_v:off_v + 64])
                        pT = apTp.tile([128, KCH, TC], BF16, tag="pT")
                        for kc in range(KCH):
                            ps = psm.tile([128, TC], F32, tag="psmm")
                            nc.tensor.matmul(
                                ps[:],
                                kh[qoff:qoff + 64, kc // TCH,
                                   (kc % TCH) * 128:(kc % TCH) * 128 + 128],
                                q_sb[qoff:qoff + 64, qm, :], start=True, stop=True)
                            nc.scalar.activation(pT[:, kc, :], ps[:], AF.Exp)
                        po = psm.tile([128, TC], F32, tag="psmm")
                        for kc in range(KCH):
                            nc.tensor.matmul(po[:], vh2[:, h % 2, kc, :],
                                             pT[:, kc, :],
                                             start=(kc == 0), stop=(kc == KCH - 1))
                        rec = sbt.tile([64, TC], F32, tag="rec")
                        nc.vector.reciprocal(rec[:], po[64:128, :])
                        nc.vector.tensor_tensor(oT[qoff:qoff + 64, qm, :],
                                                po[0:64, :], rec[:], OP.mult)
                if l == 0:
                    dbg("oT0", oT[:])
                # --- out proj + residual + ln1
                with tc.tile_pool(name="oph", bufs=4) as oph:
                    xln = xlp.tile([128, DK, TC], F32R, tag="xln")
                    for m in range(DK):
                        wt = oph.tile([128, DK, 128], BF16, tag="wt")
                        nc.sync.dma_start(wt[:], P[f"wo{l}"][m])
                        ps = psm.tile([128, TC], F32, tag="psmm")
                        for k in range(DK):
                            nc.tensor.matmul(ps[:], wt[:, k, :], oT[:, k, :],
                                             start=(k == 0), stop=(k == DK - 1))
                        t = sbt.tile([128, TC], F32, tag="ot")
                        nc.vector.tensor_scalar_add(t[:], ps[:], lb["bo"][:, m:m + 1])
                        nc.vector.tensor_tensor(xln[:, m, :], t[:], xT[:, m, :],
                                                OP.add)
                    layer_norm_(xT, xln, lb["ln1w"], lb["ln1b"], 1e-5, bdst=xB)
        if l == 0:
            dbg("xln1_0", xT[:])

        # --- router: gate scores -> top2 masks -> capacity slots -> x scatter
        C2 = c["C2"]
        CE = E * C2
        NCH = CE // TC
        assert NCH * TC == CE
        assert C2 == 192 and CE == 3 * TC
        x_send = drp.tile([CE, D], BF16, name="xsend", tag="xsend")
        x_recv = [drp.tile([TC, D], BF16, name=f"xrecv{cc}", tag=f"xrecv{cc}")
                  for cc in range(NCH)]
        y_send = [drp.tile([TC, D], BF16, name=f"ysend{cc}", tag=f"ysend{cc}")
                  for cc in range(NCH)]
        y_recv = drp.tile([CE, D], BF16, name="yrecv", tag="yrecv")
        pos_i = cst.tile([128, 2 * TCH], I32, name=f"posi{l}", tag="c_posi")
        wsv = cst.tile([128, 2 * TCH], F32, name=f"wsv{l}", tag="c_wsv")
        with tc.tile_pool(name="rph", bufs=2) as rph:
            gwt = rph.tile([128, DK, E], F32, tag="gwt", bufs=1)
            for k in range(DK):
                nc.sync.dma_start(gwt[:, k, :], P[f"gw{l}"][k])
            base_row = rph.tile([1, E], F32R, tag="base", bufs=1)
            nc.vector.tensor_scalar_mul(base_row[:], ebase_sb[:], 0.0)
            pseb = psm.tile([128, E], F32, tag="psmm")
            nc.tensor.matmul(pseb[:], ones_m[0:1, :], ebase_sb[0:1, :],
                             start=True, stop=True)
            e64b = rph.tile([128, E], F32, tag="e64b", bufs=1)
            nc.vector.tensor_copy(e64b[:], pseb[:])
            for tm in range(TCH):
                xf = rph.tile([128, DK, 128], F32, tag="xf")
                for k in range(DK):
                    nc.vector.tensor_copy(xf[:, k, :],
                                          xT[:, k, tm * 128:(tm + 1) * 128])
                psg = psm.tile([128, E], F32, tag="psmm")
                for k in range(DK):
                    nc.tensor.matmul(psg[:], xf[:, k, :], gwt[:, k, :],
                                     start=(k == 0), stop=(k == DK - 1))
                gs = rph.tile([128, E], F32, tag="gs")
                nc.vector.tensor_tensor(gs[:], psg[:], gb[:], OP.add)
                m1 = rph.tile([128, 1], F32, tag="m1")
                nc.vector.tensor_reduce(m1[:], gs[:], AX.X, OP.max)
                mask1 = rph.tile([128, E], F32, tag="mask1")
                nc.vector.tensor_tensor(mask1[:], gs[:],
                                        m1[:].to_broadcast([128, E]), OP.is_equal)
                gs2 = rph.tile([128, E], F32, tag="gs2")
                nc.vector.tensor_scalar_mul(gs2[:], mask1[:], -1e30)
                nc.vector.tensor_tensor(gs2[:], gs2[:], gs[:], OP.add)
                m2 = rph.tile([128, 1], F32, tag="m2")
                nc.vector.tensor_reduce(m2[:], gs2[:], AX.X, OP.max)
                mask2 = rph.tile([128, E], F32, tag="mask2")
                nc.vector.tensor_tensor(mask2[:], gs2[:],
                                        m2[:].to_broadcast([128, E]), OP.is_equal)
                dm = rph.tile([128, 1], F32, tag="dm")
                nc.vector.tensor_tensor(dm[:], m2[:], m1[:], OP.subtract)
                nc.scalar.activation(dm[:], dm[:], AF.Exp)
                nc.vector.tensor_scalar_add(dm[:], dm[:], 1.0)
                w1t = rph.tile([128, 1], F32, tag="w1t")
                nc.vector.reciprocal(w1t[:], dm[:])
                nc.vector.tensor_copy(wsv[:, 2 * tm:2 * tm + 1], w1t[:])
                nc.vector.tensor_scalar(wsv[:, 2 * tm + 1:2 * tm + 2], w1t[:],
                                        -1.0, 1.0, OP.mult, OP.add)
                # combined mask -> exclusive prefix rank per expert
                me = rph.tile([128, E], F32R, tag="me")
                nc.vector.tensor_tensor(me[:], mask1[:], mask2[:], OP.add)
                pse = psm.tile([128, E], F32, tag="psmm")
                nc.tensor.matmul(pse[:], triu_sb[:], me[:], start=True, stop=True)
                psb = psm.tile([128, E], F32, tag="psmm")
                nc.tensor.matmul(psb[:], ones_m[0:1, :], base_row[0:1, :],
                                 start=True, stop=True)
                bb = rph.tile([128, E], F32, tag="bb")
                nc.vector.tensor_copy(bb[:], psb[:])
                rankg = rph.tile([128, E], F32, tag="rankg")
                nc.vector.tensor_tensor(rankg[:], pse[:], bb[:], OP.add)
                nc.vector.tensor_scalar_min(rankg[:], rankg[:], float(C2 - 1))
                # chunk id c = (r>63) + (r>127) via clamp(relu(r-k),0,1)
                c1t = rph.tile([128, E], F32, tag="c1t")
                nc.vector.tensor_scalar(c1t[:], rankg[:], -63.0, 0.0,
                                        OP.add, OP.max)
                nc.vector.tensor_scalar_min(c1t[:], c1t[:], 1.0)
                c2t = rph.tile([128, E], F32, tag="c2t")
                nc.vector.tensor_scalar(c2t[:], rankg[:], -127.0, 0.0,
                                        OP.add, OP.max)
                nc.vector.tensor_scalar_min(c2t[:], c2t[:], 1.0)
                nc.vector.tensor_tensor(c1t[:], c1t[:], c2t[:], OP.add)
                # slot = r + (TC-64)*c + 64*e
                slotf = rph.tile([128, E], F32, tag="slotf")
                nc.vector.tensor_scalar(slotf[:], c1t[:], float(TC - 64), None,
                                        OP.mult)
                nc.vector.tensor_tensor(slotf[:], slotf[:], rankg[:], OP.add)
                nc.vector.tensor_tensor(slotf[:], slotf[:], e64b[:], OP.add)
                pstt = psm.tile([1, E], F32, tag="psmm")
                nc.tensor.matmul(pstt[:], ones_m[:, 0:1], me[:],
                                 start=True, stop=True)
                nc.vector.tensor_tensor(base_row[:], base_row[:], pstt[0:1, :],
                                        OP.add)
                for j, msk in ((0, mask1), (1, mask2)):
                    tt = rph.tile([128, E], F32, tag="tt")
                    nc.vector.tensor_tensor(tt[:], msk[:], slotf[:], OP.mult)
                    posf = rph.tile([128, 1], F32, tag="posf")
                    nc.vector.tensor_reduce(posf[:], tt[:], AX.X, OP.add)
                    nc.vector.tensor_copy(pos_i[:, 2 * tm + j:2 * tm + j + 1],
                                          posf[:])
                xrow = rph.tile([128, D], BF16, tag="xrow")
                for kk in range(DK // 4):
                    ptb_ = ptb.tile([128, 4, 128], BF16, tag="ptb")
                    for k4 in range(4):
                        nc.tensor.transpose(
                            ptb_[:, k4, :],
                            xB[:, kk * 4 + k4, tm * 128:(tm + 1) * 128], identB[:])
                    nc.scalar.activation(xrow[:, kk * 512:(kk + 1) * 512], ptb_[:],
                                         AF.Copy)
                for j in range(2):
                    nc.gpsimd.indirect_dma_start(
                        out=x_send[:],
                        out_offset=bass.IndirectOffsetOnAxis(
                            ap=pos_i[:, 2 * tm + j:2 * tm + j + 1], axis=0),
                        in_=xrow[:], in_offset=None)
        if l == 0 and dbg_on:
            dbg("pos0", pos_i[:])
            dbg("wsv0", wsv[:])
            with tc.tile_pool(name="dbgp", bufs=1) as dbgp:
                xs0 = dbgp.tile([128, D], BF16, tag="xs0")
                nc.sync.dma_start(xs0[:], x_send[0:128, :])
                dbg("xsend00", xs0[:])
        for cc in range(NCH):
            nc.gpsimd.collective_compute(
                "AllToAll", OP.bypass, replica_groups=GRP_ALL,
                ins=[x_send[cc * TC:(cc + 1) * TC, :]], outs=[x_recv[cc][:]])

        # --- ds mlp (local tokens; hides the x AllToAll)
        dsT_pool = tc.tile_pool(name="dsT", bufs=1)
        dsTp = dsT_pool.__enter__()
        dsT = dsTp.tile([128, DK, TC], F32, tag="dsT")
        with (
            tc.tile_pool(name="dph", bufs=1) as dph,
            tc.tile_pool(name="dphw", bufs=3) as dphw,
        ):
            gu = dph.tile([128, FK, TC], BF16, tag="gu")
            for m in range(FK):
                wtg = dphw.tile([128, DK, 128], BF16, tag="wt")
                nc.sync.dma_start(wtg[:], P[f"wg{l}"][m])
                psg = psm.tile([128, TC], F32, tag="psmm")
                for k in range(DK):
                    nc.tensor.matmul(psg[:], wtg[:, k, :], xB[:, k, :],
                                     start=(k == 0), stop=(k == DK - 1))
                sg = sbt.tile([128, TC], F32, tag="sg")
                nc.scalar.activation(sg[:], psg[:], AF.Sigmoid)
                nc.vector.tensor_tensor(sg[:], sg[:], psg[:], OP.mult)
                wtu = dphw.tile([128, DK, 128], BF16, tag="wt")
                nc.sync.dma_start(wtu[:], P[f"wu{l}"][m])
                psu = psm.tile([128, TC], F32, tag="psmm")
                for k in range(DK):
                    nc.tensor.matmul(psu[:], wtu[:, k, :], xB[:, k, :],
                                     start=(k == 0), stop=(k == DK - 1))
                nc.vector.tensor_tensor(gu[:, m, :], sg[:], psu[:], OP.mult)
            for m in range(DK):
                wtd = dphw.tile([128, FK, 128], BF16, tag="wtd", bufs=2)
                nc.sync.dma_start(wtd[:], P[f"wd{l}"][m])
                psd = psm.tile([128, TC], F32, tag="psmm")
                for k in range(FK):
                    nc.tensor.matmul(psd[:], wtd[:, k, :], gu[:, k, :],
                                     start=(k == 0), stop=(k == FK - 1))
                nc.vector.tensor_copy(dsT[:, m, :], psd[:])
        if l == 0:
            dbg("dsT0", dsT[:])

        # --- expert pass over routed tokens only, chunk-pipelined
        with (
            tc.tile_pool(name="mph", bufs=2) as mph,
            tc.tile_pool(name="mphh", bufs=1) as mphh,
            tc.tile_pool(name="mphw", bufs=3) as mphw,
            tc.tile_pool(name="mphr", bufs=3) as mphr,
        ):
            for ch in range(NCH):
                co = ch * TC
                xeT = mph.tile([128, DK, TC], BF16, tag="xeT")
                for rt in range(TC // 128):
                    xrt = mphr.tile([128, D], BF16, tag="xrt")
                    nc.sync.dma_start(xrt[:],
                                      x_recv[ch][rt * 128:(rt + 1) * 128, :])
                    for kk in range(DK // 4):
                        ptb_ = ptb.tile([128, 4, 128], BF16, tag="ptb")
                        for k4 in range(4):
                            k = kk * 4 + k4
                            nc.tensor.transpose(ptb_[:, k4, :],
                                                xrt[:, k * 128:(k + 1) * 128],
                                                identB[:])
                        nc.scalar.activation(
                            xeT[:, kk * 4:(kk + 1) * 4, rt * 128:(rt + 1) * 128],
                            ptb_[:], AF.Copy)
                hTc = mphh.tile([128, FK, TC], BF16, tag="hTc")
                for m in range(FK):
                    wt1 = mphw.tile([128, DK, 128], BF16, tag="wt")
                    nc.sync.dma_start(wt1[:], P[f"w1{l}"][m])
                    ps = psm.tile([128, TC], F32, tag="psmm")
                    for k in range(DK):
                        nc.tensor.matmul(ps[:], wt1[:, k, :], xeT[:, k, :],
                                         start=(k == 0), stop=(k == DK - 1))
                    nc.scalar.activation(hTc[:, m, :], ps[:], AF.Relu,
                                         bias=lb["b1"][:, m:m + 1])
                yTc = mphh.tile([128, DK, TC], BF16, tag="yTc")
                for m in range(DK):
                    wt2 = mphw.tile([128, FK, 128], BF16, tag="wtd", bufs=2)
                    nc.sync.dma_start(wt2[:], P[f"w2{l}"][m])
                    ps = psm.tile([128, TC], F32, tag="psmm")
                    for k in range(FK):
                        nc.tensor.matmul(ps[:], wt2[:, k, :], hTc[:, k, :],
                                         start=(k == 0), stop=(k == FK - 1))
                    nc.vector.tensor_scalar_add(yTc[:, m, :], ps[:],
                                                lb["b2"][:, m:m + 1])
                for rt in range(TC // 128):
                    yrt = mphr.tile([128, D], BF16, tag="yrt")
                    for kk in range(DK // 4):
                        ptb_ = ptb.tile([128, 4, 128], BF16, tag="ptb")
                        for k4 in range(4):
                            k = kk * 4 + k4
                            nc.tensor.transpose(ptb_[:, k4, :],
                                                yTc[:, k, rt * 128:(rt + 1) * 128],
                                                identB[:])
                        nc.scalar.activation(yrt[:, kk * 512:(kk + 1) * 512],
                                             ptb_[:], AF.Copy)
                    nc.sync.dma_start(
                        y_send[ch][rt * 128:(rt + 1) * 128, :], yrt[:])
                nc.gpsimd.collective_compute(
                    "AllToAll", OP.bypass, replica_groups=GRP_ALL,
                    ins=[y_send[ch][:]],
                    outs=[y_recv[co:co + TC, :]])

        # --- combine + ln2 (gather own tokens' two expert rows)
        xln2 = xlp.tile([128, DK, TC], F32R, tag="xln")
        with tc.tile_pool(name="cmb", bufs=2) as cmb:
            for tm in range(TCH):
                g1 = cmb.tile([128, D], BF16, tag="g1")
                nc.gpsimd.indirect_dma_start(
                    out=g1[:], out_offset=None, in_=y_recv[:],
                    in_offset=bass.IndirectOffsetOnAxis(
                        ap=pos_i[:, 2 * tm:2 * tm + 1], axis=0))
                g2 = cmb.tile([128, D], BF16, tag="g2")
                nc.gpsimd.indirect_dma_start(
                    out=g2[:], out_offset=None, in_=y_recv[:],
                    in_offset=bass.IndirectOffsetOnAxis(
                        ap=pos_i[:, 2 * tm + 1:2 * tm + 2], axis=0))
                yc = cmb.tile([128, D], F32, tag="yc")
                t2 = cmb.tile([128, D], F32, tag="t2")
                nc.vector.tensor_scalar_mul(yc[:], g1[:], wsv[:, 2 * tm:2 * tm + 1])
                nc.vector.tensor_scalar_mul(t2[:], g2[:],
                                            wsv[:, 2 * tm + 1:2 * tm + 2])
                nc.vector.tensor_tensor(yc[:], yc[:], t2[:], OP.add)
                for k in range(DK):
                    ptd = ptr.tile([128, 128], F32, tag="ptr")
                    nc.tensor.transpose(ptd[:], yc[:, k * 128:(k + 1) * 128],
                                        ident[:])
                    mo = sbt.tile([128, 128], F32, tag="mo128")
                    nc.vector.tensor_tensor(mo[:], ptd[:],
                                            dsT[:, k, tm * 128:(tm + 1) * 128],
                                            OP.add)
                    nc.vector.tensor_scalar_mul(mo[:], mo[:], 0.5)
                    nc.vector.tensor_tensor(xln2[:, k, tm * 128:(tm + 1) * 128],
                                            mo[:], xT[:, k, tm * 128:(tm + 1) * 128],
                                            OP.add)
        layer_norm_(xT, xln2, lb["ln2w"], lb["ln2b"], 1e-5, bdst=xB)
        dsT_pool.__exit__(None, None, None)

    dbg("xfinal", xT[:])
    # ---------------- final rms + allgather + lm_head
    rmsw = cst.tile([128, DK], F32, name="rmsw_sb")
    nc.sync.dma_start(rmsw[:], P["rmsw"][:])
    LMDT0 = BF16 if c.get("lm_bf16", True) else F32R
    xf_in = drp.tile([128, DK, TC], LMDT0, name="xfin")
    xf_all = drp.tile([NC, 128, DK, TC], LMDT0, name="xfall", addr_space="Shared")
    xr = xlp.tile([128, DK, TC], F32R, tag="xln")
    layer_norm_(xr, xT, rmsw, None, 1e-6, skip_mean=True)
    LMDT = BF16 if c.get("lm_bf16", True) else F32R
    xrb = xlp.tile([128, DK, TC], LMDT, tag="xrb")
    for k in range(DK):
        nc.vector.tensor_copy(xrb[:, k, :], xr[:, k, :])
    nc.sync.dma_start(xf_in[:], xrb[:])
    nc.gpsimd.collective_compute(
        "AllGather", OP.bypass, replica_groups=GRP_ALL,
        ins=[xf_in[:]], outs=[xf_all[:]])
    with (
        tc.tile_pool(name="lph", bufs=2) as lph,
        tc.tile_pool(name="lphw", bufs=4) as lphw,
    ):
        for n in range(NC):
            xfn = lph.tile([128, DK, TC], LMDT0, tag="xan")
            nc.sync.dma_start(xfn[:], xf_all[n])
            for m in range(VCK):
                wt = lphw.tile([128, DK, 128], LMDT0, tag="wt")
                nc.sync.dma_start(wt[:], P["embT"][m])
                ps = psm.tile([128, TC], F32, tag="psmm")
                for k in range(DK):
                    nc.tensor.matmul(ps[:], wt[:, k, :], xfn[:, k, :],
                                     start=(k == 0), stop=(k == DK - 1))
                lo = sbt.tile([128, TC], F32, tag="lo")
                nc.vector.tensor_copy(lo[:], ps[:])
                rows = min(128, VC - m * 128)
                nc.sync.dma_start(
                    OUT[m * 128:m * 128 + rows, n * TC:(n + 1) * TC], lo[:rows, :])

    es.close()


# ---------------------------------------------------------------- runner

def run_model(inputs, cfg, nc=None):
    c = derived(cfg)
    in_maps = prep_in_maps(inputs, cfg)
    if nc is None:
        nc = build_nc(cfg)
    res = run_bass_kernel_spmd(nc, in_maps, core_ids=list(range(c["NC"])))
    return assemble_logits(res.results, cfg), nc


# ---------------------------------------------------------------- entry point

_NC_CACHE = None


def kernel(**inputs):
    """Full-model forward on 8 trn2 cores. inputs as in reference.setup_inputs()."""
    global _NC_CACHE
    import numpy as _np
    inputs = {k: _np.asarray(v) for k, v in inputs.items()}
    if _NC_CACHE is None:
        _NC_CACHE = build_nc(FULL_CFG)
    in_maps = prep_in_maps(inputs, FULL_CFG)
    res = run_bass_kernel_spmd(_NC_CACHE, in_maps,
                               core_ids=list(range(FULL_CFG["NC"])))
    return assemble_logits(res.results, FULL_CFG)



# revision 33
# speedup vs baseline: 1.0805x; 1.0805x over previous
"""Bass/Tile kernel for nn_DeepseekV3MLPMoEModel on 8 trn2 cores.

Sharding: data-parallel over tokens (T/8 per core) for attention/MLP/lm_head
(vocab-sharded), expert-parallel for the MoE (1 expert/core, dense over all
tokens, ReduceScatter of the weighted sum).

Residual stream layout on device: xT [D(part-chunks of 128), T_loc] (f32r).
"""
import sys
sys.path.insert(0, "/opt/trn_rl_repo")
import numpy as np
import concourse.bass as bass
import concourse.mybir as mybir
import concourse.tile as tile
from concourse import bacc
from concourse.bass_utils import run_bass_kernel_spmd
from concourse.masks import make_identity

F32 = mybir.dt.float32
BF16 = mybir.dt.bfloat16
F32R = mybir.dt.float32r
I32 = mybir.dt.int32
AF = mybir.ActivationFunctionType
OP = mybir.AluOpType
AX = mybir.AxisListType

FULL_CFG = dict(B=2, S=2048, D=1024, H=16, F=2048, E=8, V=32000, L=2, NC=8, G=4,
                C2=192)
MINI_CFG = dict(B=2, S=512, D=256, H=4, F=512, E=8, V=1024, L=2, NC=8, G=4,
                C2=64)


def derived(cfg):
    c = dict(cfg)
    c["T"] = c["B"] * c["S"]
    c["TC"] = c["T"] // c["NC"]          # tokens per core
    c["TCH"] = c["TC"] // 128            # token tiles per core
    c["DK"] = c["D"] // 128              # D chunks
    c["FK"] = c["F"] // 128              # F chunks
    c["VC"] = c["V"] // c["NC"]          # vocab per core
    c["VCP"] = ((c["VC"] + 127) // 128) * 128
    c["VCK"] = c["VCP"] // 128
    c["VS"] = c["D"] // c["TC"]          # v slots per token-tile in kv pack
    c["SLOTS"] = c["DK"] + c["TCH"] * c["VS"]
    c["dh"] = c["D"] // c["H"]
    assert c["dh"] == 64
    return c


# ---------------------------------------------------------------- host prep

def lhsT_tiles(W, bf16=True):
    """W [M, K] (for out = x @ W.T) -> [M/128, 128(ki), K/128(ko), 128(mm)]."""
    import ml_dtypes
    M, K = W.shape
    Wt = np.ascontiguousarray(W.T)
    r = np.ascontiguousarray(
        Wt.reshape(K // 128, 128, M // 128, 128).transpose(2, 1, 0, 3))
    return r.astype(ml_dtypes.bfloat16) if bf16 else r


def rhs_tiles(W, bf16=False):
    """W [N, K] (used as rhs [K, N]) -> [K/128, 128, N]."""
    import ml_dtypes
    N, K = W.shape
    r = np.ascontiguousarray(W.T.reshape(K // 128, 128, N))
    return r.astype(ml_dtypes.bfloat16) if bf16 else r


def pp_cols(b):
    """b [M] -> [128, M/128]: column m holds b[m*128:(m+1)*128]."""
    return np.ascontiguousarray(b.reshape(-1, 128).T)


def prep_in_maps(inputs, cfg):
    c = derived(cfg)
    NC, L, D, E = c["NC"], c["L"], c["D"], c["E"]
    VC, VCP = c["VC"], c["VCP"]
    f32 = np.float32

    tokens = np.asarray(inputs["tokens"]).astype(np.int64).reshape(-1)  # [T]
    emb = np.asarray(inputs["emb"], f32)

    shared = {}
    for l in range(L):
        ipw = np.asarray(inputs["in_proj_w"][l], f32)     # [3D, D]
        ipb = np.asarray(inputs["in_proj_b"][l], f32)     # [3D]
        bqk = ipb[:2 * D].copy()
        bqk[:D] *= 0.125
        shared[f"wqk{l}"] = lhsT_tiles(ipw[:2 * D], bf16=True)
        shared[f"bqk{l}"] = pp_cols(bqk)
        shared[f"wv{l}"] = rhs_tiles(ipw[2 * D:], bf16=True)
        shared[f"bv{l}"] = ipb[2 * D:].reshape(1, D).copy()
        shared[f"wo{l}"] = lhsT_tiles(np.asarray(inputs["out_proj_w"][l], f32), bf16=True)
        shared[f"bo{l}"] = pp_cols(np.asarray(inputs["out_proj_b"][l], f32))
        for nm in ("ln1_w", "ln1_b", "ln2_w", "ln2_b"):
            shared[f"{nm.replace('_','')}{l}"] = pp_cols(np.asarray(inputs[nm][l], f32))
        shared[f"wg{l}"] = lhsT_tiles(np.asarray(inputs["ds_gate_w"][l], f32), bf16=True)
        shared[f"wu{l}"] = lhsT_tiles(np.asarray(inputs["ds_up_w"][l], f32), bf16=True)
        shared[f"wd{l}"] = lhsT_tiles(np.asarray(inputs["ds_down_w"][l], f32), bf16=True)
        shared[f"gw{l}"] = rhs_tiles(np.asarray(inputs["gate_w"][l], f32))
        shared[f"gb{l}"] = np.asarray(inputs["gate_b"][l], f32).reshape(1, E).copy()
    shared["rmsw"] = pp_cols(np.asarray(inputs["rms_w"], f32))
    shared["ones_mat"] = np.ones((128, 128), f32)
    import ml_dtypes
    shared["ones_bf"] = np.ones((128, 64), ml_dtypes.bfloat16)
    shared["triu"] = np.triu(np.ones((128, 128), f32), 1)
    shared["ebase"] = (np.arange(E) * 64).astype(f32).reshape(1, E)

    in_maps = []
    for core in range(NC):
        m = dict(shared)
        lo = core * VC
        m["embrows"] = emb  # replicated full table
        loc = tokens[core * (len(tokens) // NC):(core + 1) * (len(tokens) // NC)]
        m["tokidx"] = np.ascontiguousarray(
            loc.reshape(-1, 128).T.astype(np.int32))  # [128, TC/128]
        esl = np.zeros((VCP, D), f32)
        esl[:VC] = emb[lo:lo + VC]
        m["embT"] = lhsT_tiles(esl, bf16=c.get("lm_bf16", True))
        for l in range(L):
            m[f"w1{l}"] = lhsT_tiles(np.asarray(inputs["moe_w1"][l, core], f32), bf16=True)
            m[f"b1{l}"] = pp_cols(np.asarray(inputs["moe_b1"][l, core], f32))
            m[f"w2{l}"] = lhsT_tiles(np.asarray(inputs["moe_w2"][l, core], f32), bf16=True)
            m[f"b2{l}"] = pp_cols(np.asarray(inputs["moe_b2"][l, core], f32))
        in_maps.append(m)
    return in_maps


def assemble_logits(results, cfg):
    c = derived(cfg)
    B, S, V, VC = c["B"], c["S"], c["V"], c["VC"]
    out = np.empty((B, S, V), np.float32)
    for core, r in enumerate(results):
        lg = r["logits"]  # [VC, T]
        out[:, :, core * VC:(core + 1) * VC] = lg.T.reshape(B, S, VC)
    return out


# ---------------------------------------------------------------- device code

def build_nc(cfg):
    c = derived(cfg)
    L, D, E = c["L"], c["D"], c["E"]
    DK, FK = c["DK"], c["FK"]
    VC, VCK = c["VC"], c["VCK"]
    T = c["T"]

    nc = bacc.Bacc(None)
    P = {}

    def par(name, shape, dt):
        P[name] = nc.dram_tensor(name, shape, dt, kind="ExternalInput")

    par("tokidx", [128, T // (8 * 128)], I32)
    par("ones_mat", [128, 128], F32R)
    par("ones_bf", [128, 64], BF16)
    par("triu", [128, 128], F32R)
    par("ebase", [1, E], F32R)
    par("embrows", [c["V"], D], F32)
    par("embT", [VCK, 128, DK, 128], BF16 if c.get("lm_bf16", True) else F32R)
    for l in range(L):
        par(f"wqk{l}", [2 * DK, 128, DK, 128], BF16)
        par(f"bqk{l}", [128, 2 * DK], F32)
        par(f"wv{l}", [DK, 128, D], BF16)
        par(f"bv{l}", [1, D], F32R)
        par(f"wo{l}", [DK, 128, DK, 128], BF16)
        par(f"bo{l}", [128, DK], F32)
        for nm in ("ln1w", "ln1b", "ln2w", "ln2b"):
            par(f"{nm}{l}", [128, DK], F32)
        par(f"wg{l}", [FK, 128, DK, 128], BF16)
        par(f"wu{l}", [FK, 128, DK, 128], BF16)
        par(f"wd{l}", [DK, 128, FK, 128], BF16)
        par(f"gw{l}", [DK, 128, E], F32)
        par(f"gb{l}", [1, E], F32R)
        par(f"w1{l}", [FK, 128, DK, 128], BF16)
        par(f"b1{l}", [128, FK], F32)
        par(f"w2{l}", [DK, 128, FK, 128], BF16)
        par(f"b2{l}", [128, DK], F32)
    par("rmsw", [128, DK], F32)
    OUT = nc.dram_tensor("logits", [VC, T], F32, kind="ExternalOutput")

    with tile.TileContext(nc) as tc:
        _emit(nc, tc, P, OUT, c)
    nc.compile()
    return nc


def _emit(nc, tc, P, OUT, c):
    NC, L, D, H, F, E = c["NC"], c["L"], c["D"], c["H"], c["F"], c["E"]
    TC, TCH, DK, FK = c["TC"], c["TCH"], c["DK"], c["FK"]
    VC, VCK, VS, SLOTS = c["VC"], c["VCK"], c["VS"], c["SLOTS"]
    G, T = c["G"], c["T"]
    KCH = G * TCH
    TK = T // 128
    NDN = max(1, D // 512)
    NW = min(512, D)
    GRP_KV = [list(range(g * G, (g + 1) * G)) for g in range(NC // G)]
    GRP_ALL = [list(range(NC))]

    from contextlib import ExitStack
    es = ExitStack()
    cst = es.enter_context(tc.tile_pool(name="cst", bufs=1))
    sbt = es.enter_context(tc.tile_pool(name="sbt", bufs=2))
    lnp = es.enter_context(tc.tile_pool(name="lnp", bufs=2))
    xlp = es.enter_context(tc.tile_pool(name="xlp", bufs=1))
    psm = es.enter_context(tc.tile_pool(name="psm", bufs=3, space="PSUM"))
    pst = es.enter_context(tc.tile_pool(name="pst", bufs=2, space="PSUM"))
    ptr = es.enter_context(tc.tile_pool(name="ptr", bufs=1, space="PSUM"))
    drp = es.enter_context(tc.tile_pool(name="drp", bufs=1, space="DRAM"))

    dbg_on = c.get("debug", False)

    def dbg(name, ap):
        if not dbg_on:
            return
        t = nc.dram_tensor(f"dbg_{name}", list(ap.shape), ap.dtype,
                           kind="ExternalOutput")
        nc.sync.dma_start(t[:], ap)

    ident = cst.tile([128, 128], F32, name="ident")
    make_identity(nc, ident)
    identB = cst.tile([128, 128], BF16, name="identB")
    nc.vector.tensor_copy(identB[:], ident[:])
    ones_m = cst.tile([128, 128], F32R, name="ones_m")
    nc.sync.dma_start(ones_m[:], P["ones_mat"][:])
    triu_sb = cst.tile([128, 128], F32R, name="triu_sb")
    nc.sync.dma_start(triu_sb[:], P["triu"][:])
    ebase_sb = cst.tile([1, E], F32R, name="ebase_sb")
    nc.sync.dma_start(ebase_sb[:], P["ebase"][:])
    ptb = es.enter_context(tc.tile_pool(name="ptb", bufs=2, space="PSUM"))
    eps5 = cst.tile([128, 1], F32, name="eps5")
    nc.gpsimd.memset(eps5[:], 1e-5)
    eps6 = cst.tile([128, 1], F32, name="eps6")
    nc.gpsimd.memset(eps6[:], 1e-6)
    xT = cst.tile([128, DK, TC], F32R, name="xT")
    xB = cst.tile([128, DK, TC], BF16, name="xB")
    
    KCH_ = G * TCH


    # ---------------- embedding: gather own tokens from replicated table
    with tc.tile_pool(name="emb_ph", bufs=3) as ph:
        idx_sb = ph.tile([128, TCH], I32, name="idx_sb", bufs=1)
        nc.sync.dma_start(idx_sb[:], P["tokidx"][:])
        sqrt_d = float(np.sqrt(c["D"]))
        for tm in range(TCH):
            ge = ph.tile([128, D], F32, tag="ge")
            nc.gpsimd.indirect_dma_start(
                out=ge[:], out_offset=None, in_=P["embrows"][:],
                in_offset=bass.IndirectOffsetOnAxis(ap=idx_sb[:, tm:tm + 1], axis=0))
            for k in range(DK):
                pt = ptr.tile([128, 128], F32, tag="ptr")
                nc.tensor.transpose(pt[:], ge[:, k * 128:(k + 1) * 128], ident[:])
                nc.scalar.activation(xT[:, k, tm * 128:(tm + 1) * 128], pt[:],
                                     AF.Copy, scale=sqrt_d)
                nc.vector.tensor_copy(xB[:, k, tm * 128:(tm + 1) * 128],
                                      xT[:, k, tm * 128:(tm + 1) * 128])
    dbg("x0T", xT[:])

    # ---------------- LN helper (matmul stats, replicated across partitions)
    def layer_norm_(dst, src, wcols, bcols, eps, skip_mean=False, bdst=None):
        eps = eps5[:, 0:1] if eps == 1e-5 else eps6[:, 0:1]
        ps1 = None if skip_mean else pst.tile([128, TC], F32, tag="pstat")
        ps2 = pst.tile([128, TC], F32, tag="pstat")
        for k in range(DK):
            sq = lnp.tile([128, TC], F32R, tag="sq")
            nc.vector.tensor_tensor(sq[:], src[:, k, :], src[:, k, :], OP.mult)
            if not skip_mean:
                nc.tensor.matmul(ps1[:], ones_m[:], src[:, k, :],
                                 start=(k == 0), stop=(k == DK - 1))
            nc.tensor.matmul(ps2[:], ones_m[:], sq[:],
                             start=(k == 0), stop=(k == DK - 1))
        e2 = lnp.tile([128, TC], F32, tag="stmp")
        nc.scalar.activation(e2[:], ps2[:], AF.Copy, scale=1.0 / c["D"])
        if not skip_mean:
            mu = lnp.tile([128, TC], F32, tag="smu", bufs=1)
            nc.scalar.activation(mu[:], ps1[:], AF.Copy, scale=1.0 / c["D"])
            var = lnp.tile([128, TC], F32, tag="stmp")
            nc.vector.tensor_tensor(var[:], mu[:], mu[:], OP.mult)
            nc.vector.tensor_tensor(var[:], e2[:], var[:], OP.subtract)
        else:
            var = e2
        sd = lnp.tile([128, TC], F32, tag="stmp")
        nc.scalar.activation(sd[:], var[:], AF.Sqrt, bias=eps)
        rstd = lnp.tile([128, TC], F32, tag="srstd", bufs=1)
        nc.vector.reciprocal(rstd[:], sd[:])
        for k in range(DK):
            t1 = lnp.tile([128, TC], F32, tag="lnt")
            if not skip_mean:
                nc.vector.tensor_tensor(t1[:], src[:, k, :], mu[:], OP.subtract)
                nc.vector.tensor_tensor(t1[:], t1[:], rstd[:], OP.mult)
            else:
                nc.vector.tensor_tensor(t1[:], src[:, k, :], rstd[:], OP.mult)
            if bcols is not None:
                nc.vector.tensor_scalar(dst[:, k, :], t1[:],
                                        wcols[:, k:k + 1], bcols[:, k:k + 1],
                                        OP.mult, OP.add)
            else:
                nc.vector.tensor_scalar_mul(dst[:, k, :], t1[:], wcols[:, k:k + 1])
            if bdst is not None:
                nc.scalar.activation(bdst[:, k, :], dst[:, k, :], AF.Copy)

    # ---------------- layers
    for l in range(L):
        lb = {}
        for nm in ("bqk", "bo", "ln1w", "ln1b", "ln2w", "ln2b", "b1", "b2"):
            w = P[f"{nm}{l}"].shape[1]
            t = cst.tile([128, w], F32, name=f"{nm}{l}_sb", tag=f"c_{nm}")
            nc.sync.dma_start(t[:], P[f"{nm}{l}"][:])
            lb[nm] = t
        bv1 = cst.tile([1, D], F32R, name=f"bv1_{l}", tag="c_bv1")
        nc.sync.dma_start(bv1[:], P[f"bv{l}"][:])
        bv = cst.tile([128, D], F32, name=f"bv{l}_sb", tag="c_bv")
        for dn in range(NDN):
            psb = psm.tile([128, NW], F32, tag="psmm")
            nc.tensor.matmul(psb[:], ones_m[0:1, :],
                             bv1[0:1, dn * NW:(dn + 1) * NW], start=True, stop=True)
            nc.vector.tensor_copy(bv[:, dn * NW:(dn + 1) * NW], psb[:])
        gb1 = cst.tile([1, E], F32R, name=f"gb1_{l}", tag="c_gb1")
        nc.sync.dma_start(gb1[:], P[f"gb{l}"][:])
        psgb = psm.tile([128, E], F32, tag="psmm")
        nc.tensor.matmul(psgb[:], ones_m[0:1, :], gb1[0:1, :], start=True, stop=True)
        gb = cst.tile([128, E], F32, name=f"gb{l}_sb", tag="c_gb")
        nc.vector.tensor_copy(gb[:], psgb[:])

        assert NDN == 2 and NW == TC and VS == 2
        kv_ink = [drp.tile([128, TC], BF16, name=f"kvink{m}", tag=f"kvink{m}")
                  for m in range(DK)]
        kv_allk = [drp.tile([G, 128, TC], BF16, name=f"kvallk{m}",
                            tag=f"kvallk{m}") for m in range(DK)]
        kv_inv = [drp.tile([TCH, 128, TC], BF16, name=f"kvinv{dn}",
                           tag=f"kvinv{dn}") for dn in range(NDN)]
        kv_allv = [drp.tile([G, TCH, 128, TC], BF16, name=f"kvallv{dn}",
                            tag=f"kvallv{dn}") for dn in range(NDN)]

        # --- qkv phase: k first (per-slot gathers pipeline), then q, then v
        with tc.tile_pool(name="qp", bufs=1) as qp:
            q_sb = qp.tile([128, DK, TC], BF16, tag="q_sb")
            with (
                tc.tile_pool(name="qphw", bufs=4) as qphw,
                tc.tile_pool(name="qphk", bufs=2) as qphk,
                tc.tile_pool(name="qpv", bufs=1) as qpv,
            ):
                wv0 = qpv.tile([128, DK, NW], BF16, tag="wv0")
                wv1 = qpv.tile([128, DK, NW], BF16, tag="wv1")
                for dn, wv in enumerate((wv0, wv1)):
                    for k in range(DK):
                        nc.sync.dma_start(wv[:, k, :],
                                          P[f"wv{l}"][k, :, dn * NW:(dn + 1) * NW])
                for m in range(2 * DK):
                    mm = (m + DK) % (2 * DK)        # k chunks first
                    wt = qphw.tile([128, DK, 128], BF16, tag="wt")
                    nc.sync.dma_start(wt[:], P[f"wqk{l}"][mm])
                    ps = psm.tile([128, TC], F32, tag="psmm")
                    for k in range(DK):
                        nc.tensor.matmul(ps[:], wt[:, k, :], xB[:, k, :],
                                         start=(k == 0), stop=(k == DK - 1))
                    if mm < DK:
                        nc.scalar.activation(q_sb[:, mm, :], ps[:], AF.Identity,
                                             scale=0.125, bias=lb["bqk"][:, mm:mm + 1])
                    else:
                        kt = qphk.tile([128, TC], BF16, tag="kt")
                        nc.scalar.activation(kt[:], ps[:], AF.Identity,
                                             bias=lb["bqk"][:, mm:mm + 1])
                        nc.sync.dma_start(kv_ink[mm - DK][:], kt[:])
                        nc.gpsimd.collective_compute(
                            "AllGather", OP.bypass, replica_groups=GRP_KV,
                            ins=[kv_ink[mm - DK][:]], outs=[kv_allk[mm - DK][:]])
                for dn, wv in enumerate((wv0, wv1)):
                    for tm in range(TCH):
                        ps = psm.tile([128, NW], F32, tag="psmm")
                        for k in range(DK):
                            nc.tensor.matmul(ps[:], xB[:, k, tm * 128:(tm + 1) * 128],
                                             wv[:, k, :],
                                             start=(k == 0), stop=(k == DK - 1))
                        vt = qphk.tile([128, NW], BF16, tag="vt")
                        nc.vector.tensor_tensor(
                            vt[:], ps[:], bv[:, dn * NW:(dn + 1) * NW], OP.add)
                        nc.sync.dma_start(kv_inv[dn][tm], vt[:])
                    nc.gpsimd.collective_compute(
                        "AllGather", OP.bypass, replica_groups=GRP_KV,
                        ins=[kv_inv[dn][:]], outs=[kv_allv[dn][:]])
            if l == 0:
                dbg("q0", q_sb[:])

            # --- attention (q_sb in scope)
            with tc.tile_pool(name="aoT", bufs=1) as aoTp:
                oT = aoTp.tile([128, DK, TC], BF16, tag="oT")
                vh2 = aoTp.tile([128, 2, KCH, 128], BF16, tag="vh2")
                for b_ in range(2):
                    for kc_ in range(KCH):
                        nc.sync.dma_start(vh2[:, b_, kc_, 64:128],
                                          P["ones_bf"][:, 0:64])
                with (
                    tc.tile_pool(name="aph", bufs=2) as aph,
                    tc.tile_pool(name="apT", bufs=2) as apTp,
                ):
                    for h in range(H):
                        qm, qoff = h // 2, 64 * (h % 2)
                        kh = aph.tile([128, G, TC], BF16, tag="kh")
                        for g in range(G):
                            nc.sync.dma_start(kh[qoff:qoff + 64, g, :],
                                              kv_allk[qm][g, qoff:qoff + 64, :])
                        s_v, off_v = h // (TC // 64), (64 * h) % TC
                        for g in range(G):
                            for tm in range(TCH):
                                nc.sync.dma_start(
                                    vh2[:, h % 2, g * TCH + tm, 0:64],
                                    kv_allv[s_v][g, tm, :, off_v:off_v + 64])
                        pT = apTp.tile([128, KCH, TC], BF16, tag="pT")
                        for kc in range(KCH):
                            ps = psm.tile([128, TC], F32, tag="psmm")
                            nc.tensor.matmul(
                                ps[:],
                                kh[qoff:qoff + 64, kc // TCH,
                                   (kc % TCH) * 128:(kc % TCH) * 128 + 128],
                                q_sb[qoff:qoff + 64, qm, :], start=True, stop=True)
                            nc.scalar.activation(pT[:, kc, :], ps[:], AF.Exp)
                        po = psm.tile([128, TC], F32, tag="psmm")
                        for kc in range(KCH):
                            nc.tensor.matmul(po[:], vh2[:, h % 2, kc, :],
                                             pT[:, kc, :],
                                             start=(kc == 0), stop=(kc == KCH - 1))
                        rec = sbt.tile([64, TC], F32, tag="rec")
                        nc.vector.reciprocal(rec[:], po[64:128, :])
                        nc.vector.tensor_tensor(oT[qoff:qoff + 64, qm, :],
                                                po[0:64, :], rec[:], OP.mult)
                if l == 0:
                    dbg("oT0", oT[:])
                # --- out proj + residual + ln1
                with tc.tile_pool(name="oph", bufs=4) as oph:
                    xln = xlp.tile([128, DK, TC], F32R, tag="xln")
                    for m in range(DK):
                        wt = oph.tile([128, DK, 128], BF16, tag="wt")
                        nc.sync.dma_start(wt[:], P[f"wo{l}"][m])
                        ps = psm.tile([128, TC], F32, tag="psmm")
                        for k in range(DK):
                            nc.tensor.matmul(ps[:], wt[:, k, :], oT[:, k, :],
                                             start=(k == 0), stop=(k == DK - 1))
                        t = sbt.tile([128, TC], F32, tag="ot")
                        nc.vector.tensor_scalar_add(t[:], ps[:], lb["bo"][:, m:m + 1])
                        nc.vector.tensor_tensor(xln[:, m, :], t[:], xT[:, m, :],
                                                OP.add)
                    layer_norm_(xT, xln, lb["ln1w"], lb["ln1b"], 1e-5, bdst=xB)
        if l == 0:
            dbg("xln1_0", xT[:])

        # --- router: gate scores -> top2 masks -> capacity slots -> x scatter
        C2 = c["C2"]
        CE = E * C2
        NCH = CE // TC
        assert NCH * TC == CE
        assert C2 == 192 and CE == 3 * TC
        x_send = drp.tile([CE, D], BF16, name="xsend", tag="xsend")
        x_recv = [drp.tile([TC, D], BF16, name=f"xrecv{cc}", tag=f"xrecv{cc}")
                  for cc in range(NCH)]
        y_send = [drp.tile([TC, D], BF16, name=f"ysend{cc}", tag=f"ysend{cc}")
                  for cc in range(NCH)]
        y_recv = drp.tile([CE, D], BF16, name="yrecv", tag="yrecv")
        pos_i = cst.tile([128, 2 * TCH], I32, name=f"posi{l}", tag="c_posi")
        wsv = cst.tile([128, 2 * TCH], F32, name=f"wsv{l}", tag="c_wsv")
        with tc.tile_pool(name="rph", bufs=2) as rph:
            gwt = rph.tile([128, DK, E], F32, tag="gwt", bufs=1)
            for k in range(DK):
                nc.sync.dma_start(gwt[:, k, :], P[f"gw{l}"][k])
            base_row = rph.tile([1, E], F32R, tag="base", bufs=1)
            nc.vector.tensor_scalar_mul(base_row[:], ebase_sb[:], 0.0)
            pseb = psm.tile([128, E], F32, tag="psmm")
            nc.tensor.matmul(pseb[:], ones_m[0:1, :], ebase_sb[0:1, :],
                             start=True, stop=True)
            e64b = rph.tile([128, E], F32, tag="e64b", bufs=1)
            nc.vector.tensor_copy(e64b[:], pseb[:])
            for tm in range(TCH):
                xf = rph.tile([128, DK, 128], F32, tag="xf")
                for k in range(DK):
                    nc.vector.tensor_copy(xf[:, k, :],
                                          xT[:, k, tm * 128:(tm + 1) * 128])
                psg = psm.tile([128, E], F32, tag="psmm")
                for k in range(DK):
                    nc.tensor.matmul(psg[:], xf[:, k, :], gwt[:, k, :],
                                     start=(k == 0), stop=(k == DK - 1))
                gs = rph.tile([128, E], F32, tag="gs")
                nc.vector.tensor_tensor(gs[:], psg[:], gb[:], OP.add)
                m1 = rph.tile([128, 1], F32, tag="m1")
                nc.vector.tensor_reduce(m1[:], gs[:], AX.X, OP.max)
                mask1 = rph.tile([128, E], F32, tag="mask1")
                nc.vector.tensor_tensor(mask1[:], gs[:],
                                        m1[:].to_broadcast([128, E]), OP.is_equal)
                gs2 = rph.tile([128, E], F32, tag="gs2")
                nc.vector.tensor_scalar_mul(gs2[:], mask1[:], -1e30)
                nc.vector.tensor_tensor(gs2[:], gs2[:], gs[:], OP.add)
                m2 = rph.tile([128, 1], F32, tag="m2")
                nc.vector.tensor_reduce(m2[:], gs2[:], AX.X, OP.max)
                mask2 = rph.tile([128, E], F32, tag="mask2")
                nc.vector.tensor_tensor(mask2[:], gs2[:],
                                        m2[:].to_broadcast([128, E]), OP.is_equal)
                dm = rph.tile([128, 1], F32, tag="dm")
                nc.vector.tensor_tensor(dm[:], m2[:], m1[:], OP.subtract)
                nc.scalar.activation(dm[:], dm[:], AF.Exp)
                nc.vector.tensor_scalar_add(dm[:], dm[:], 1.0)
                w1t = rph.tile([128, 1], F32, tag="w1t")
                nc.vector.reciprocal(w1t[:], dm[:])
                nc.vector.tensor_copy(wsv[:, 2 * tm:2 * tm + 1], w1t[:])
                nc.vector.tensor_scalar(wsv[:, 2 * tm + 1:2 * tm + 2], w1t[:],
                                        -1.0, 1.0, OP.mult, OP.add)
                # combined mask -> exclusive prefix rank per expert
                me = rph.tile([128, E], F32R, tag="me")
                nc.vector.tensor_tensor(me[:], mask1[:], mask2[:], OP.add)
                pse = psm.tile([128, E], F32, tag="psmm")
                nc.tensor.matmul(pse[:], triu_sb[:], me[:], start=True, stop=True)
                psb = psm.tile([128, E], F32, tag="psmm")
                nc.tensor.matmul(psb[:], ones_m[0:1, :], base_row[0:1, :],
                                 start=True, stop=True)
                bb = rph.tile([128, E], F32, tag="bb")
                nc.vector.tensor_copy(bb[:], psb[:])
                rankg = rph.tile([128, E], F32, tag="rankg")
                nc.vector.tensor_tensor(rankg[:], pse[:], bb[:], OP.add)
                nc.vector.tensor_scalar_min(rankg[:], rankg[:], float(C2 - 1))
                # chunk id c = (r>63) + (r>127) via clamp(relu(r-k),0,1)
                c1t = rph.tile([128, E], F32, tag="c1t")
                nc.vector.tensor_scalar(c1t[:], rankg[:], -63.0, 0.0,
                                        OP.add, OP.max)
                nc.vector.tensor_scalar_min(c1t[:], c1t[:], 1.0)
                c2t = rph.tile([128, E], F32, tag="c2t")
                nc.vector.tensor_scalar(c2t[:], rankg[:], -127.0, 0.0,
                                        OP.add, OP.max)
                nc.vector.tensor_scalar_min(c2t[:], c2t[:], 1.0)
                nc.vector.tensor_tensor(c1t[:], c1t[:], c2t[:], OP.add)
                # slot = r + (TC-64)*c + 64*e
                slotf = rph.tile([128, E], F32, tag="slotf")
                nc.vector.tensor_scalar(slotf[:], c1t[:], float(TC - 64), None,
                                        OP.mult)
                nc.vector.tensor_tensor(slotf[:], slotf[:], rankg[:], OP.add)
                nc.vector.tensor_tensor(slotf[:], slotf[:], e64b[:], OP.add)
                pstt = psm.tile([1, E], F32, tag="psmm")
                nc.tensor.matmul(pstt[:], ones_m[:, 0:1], me[:],
                                 start=True, stop=True)
                nc.vector.tensor_tensor(base_row[:], base_row[:], pstt[0:1, :],
                                        OP.add)
                for j, msk in ((0, mask1), (1, mask2)):
                    tt = rph.tile([128, E], F32, tag="tt")
                    nc.vector.tensor_tensor(tt[:], msk[:], slotf[:], OP.mult)
                    posf = rph.tile([128, 1], F32, tag="posf")
                    nc.vector.tensor_reduce(posf[:], tt[:], AX.X, OP.add)
                    nc.vector.tensor_copy(pos_i[:, 2 * tm + j:2 * tm + j + 1],
                                          posf[:])
                xrow = rph.tile([128, D], BF16, tag="xrow")
                for kk in range(DK // 4):
                    ptb_ = ptb.tile([128, 4, 128], BF16, tag="ptb")
                    for k4 in range(4):
                        nc.tensor.transpose(
                            ptb_[:, k4, :],
                            xB[:, kk * 4 + k4, tm * 128:(tm + 1) * 128], identB[:])
                    nc.scalar.activation(xrow[:, kk * 512:(kk + 1) * 512], ptb_[:],
                                         AF.Copy)
                for j in range(2):
                    nc.gpsimd.indirect_dma_start(
                        out=x_send[:],
                        out_offset=bass.IndirectOffsetOnAxis(
                            ap=pos_i[:, 2 * tm + j:2 * tm + j + 1], axis=0),
                        in_=xrow[:], in_offset=None)
        if l == 0 and dbg_on:
            dbg("pos0", pos_i[:])
            dbg("wsv0", wsv[:])
            with tc.tile_pool(name="dbgp", bufs=1) as dbgp:
                xs0 = dbgp.tile([128, D], BF16, tag="xs0")
                nc.sync.dma_start(xs0[:], x_send[0:128, :])
                dbg("xsend00", xs0[:])
        for cc in range(NCH):
            nc.gpsimd.collective_compute(
                "AllToAll", OP.bypass, replica_groups=GRP_ALL,
                ins=[x_send[cc * TC:(cc + 1) * TC, :]], outs=[x_recv[cc][:]])

        # --- ds mlp (local tokens; hides the x AllToAll)
        dsT_pool = tc.tile_pool(name="dsT", bufs=1)
        dsTp = dsT_pool.__enter__()
        dsT = dsTp.tile([128, DK, TC], F32, tag="dsT")
        with (
            tc.tile_pool(name="dph", bufs=1) as dph,
            tc.tile_pool(name="dphw", bufs=3) as dphw,
        ):
            gu = dph.tile([128, FK, TC], BF16, tag="gu")
            for m in range(FK):
                wtg = dphw.tile([128, DK, 128], BF16, tag="wt")
                nc.sync.dma_start(wtg[:], P[f"wg{l}"][m])
                psg = psm.tile([128, TC], F32, tag="psmm")
                for k in range(DK):
                    nc.tensor.matmul(psg[:], wtg[:, k, :], xB[:, k, :],
                                     start=(k == 0), stop=(k == DK - 1))
                sg = sbt.tile([128, TC], F32, tag="sg")
                nc.scalar.activation(sg[:], psg[:], AF.Sigmoid)
                nc.vector.tensor_tensor(sg[:], sg[:], psg[:], OP.mult)
                wtu = dphw.tile([128, DK, 128], BF16, tag="wt")
                nc.sync.dma_start(wtu[:], P[f"wu{l}"][m])
                psu = psm.tile([128, TC], F32, tag="psmm")
                for k in range(DK):
                    nc.tensor.matmul(psu[:], wtu[:, k, :], xB[:, k, :],
                                     start=(k == 0), stop=(k == DK - 1))
                nc.vector.tensor_tensor(gu[:, m, :], sg[:], psu[:], OP.mult)
            for m in range(DK):
                wtd = dphw.tile([128, FK, 128], BF16, tag="wtd", bufs=2)
                nc.sync.dma_start(wtd[:], P[f"wd{l}"][m])
                psd = psm.tile([128, TC], F32, tag="psmm")
                for k in range(FK):
                    nc.tensor.matmul(psd[:], wtd[:, k, :], gu[:, k, :],
                                     start=(k == 0), stop=(k == FK - 1))
                nc.vector.tensor_copy(dsT[:, m, :], psd[:])
        if l == 0:
            dbg("dsT0", dsT[:])

        # --- expert pass over routed tokens only, chunk-pipelined
        with (
            tc.tile_pool(name="mph", bufs=2) as mph,
            tc.tile_pool(name="mphh", bufs=1) as mphh,
            tc.tile_pool(name="mphw", bufs=3) as mphw,
            tc.tile_pool(name="mphr", bufs=3) as mphr,
        ):
            for ch in range(NCH):
                co = ch * TC
                xeT = mph.tile([128, DK, TC], BF16, tag="xeT")
                for rt in range(TC // 128):
                    xrt = mphr.tile([128, D], BF16, tag="xrt")
                    nc.sync.dma_start(xrt[:],
                                      x_recv[ch][rt * 128:(rt + 1) * 128, :])
                    for kk in range(DK // 4):
                        ptb_ = ptb.tile([128, 4, 128], BF16, tag="ptb")
                        for k4 in range(4):
                            k = kk * 4 + k4
                            nc.tensor.transpose(ptb_[:, k4, :],
                                                xrt[:, k * 128:(k + 1) * 128],
                                                identB[:])
                        nc.scalar.activation(
                            xeT[:, kk * 4:(kk + 1) * 4, rt * 128:(rt + 1) * 128],
                            ptb_[:], AF.Copy)
                hTc = mphh.tile([128, FK, TC], BF16, tag="hTc")
                for m in range(FK):
                    wt1 = mphw.tile([128, DK, 128], BF16, tag="wt")
                    nc.sync.dma_start(wt1[:], P[f"w1{l}"][m])
                    ps = psm.tile([128, TC], F32, tag="psmm")
                    for k in range(DK):
                        nc.tensor.matmul(ps[:], wt1[:, k, :], xeT[:, k, :],
                                         start=(k == 0), stop=(k == DK - 1))
                    nc.scalar.activation(hTc[:, m, :], ps[:], AF.Relu,
                                         bias=lb["b1"][:, m:m + 1])
                yTc = mphh.tile([128, DK, TC], BF16, tag="yTc")
                for m in range(DK):
                    wt2 = mphw.tile([128, FK, 128], BF16, tag="wtd", bufs=2)
                    nc.sync.dma_start(wt2[:], P[f"w2{l}"][m])
                    ps = psm.tile([128, TC], F32, tag="psmm")
                    for k in range(FK):
                        nc.tensor.matmul(ps[:], wt2[:, k, :], hTc[:, k, :],
                                         start=(k == 0), stop=(k == FK - 1))
                    nc.vector.tensor_scalar_add(yTc[:, m, :], ps[:],
                                                lb["b2"][:, m:m + 1])
                for rt in range(TC // 128):
                    yrt = mphr.tile([128, D], BF16, tag="yrt")
                    for kk in range(DK // 4):
                        ptb_ = ptb.tile([128, 4, 128], BF16, tag="ptb")
                        for k4 in range(4):
                            k = kk * 4 + k4
                            nc.tensor.transpose(ptb_[:, k4, :],
                                                yTc[:, k, rt * 128:(rt + 1) * 128],
                                                identB[:])
                        nc.scalar.activation(yrt[:, kk * 512:(kk + 1) * 512],
                                             ptb_[:], AF.Copy)
                    nc.sync.dma_start(
                        y_send[ch][rt * 128:(rt + 1) * 128, :], yrt[:])
                nc.gpsimd.collective_compute(
                    "AllToAll", OP.bypass, replica_groups=GRP_ALL,
                    ins=[y_send[ch][:]],
                    outs=[y_recv[co:co + TC, :]])

        # --- combine + ln2 (gather own tokens' two expert rows)
        xln2 = xlp.tile([128, DK, TC], F32R, tag="xln")
        with tc.tile_pool(name="cmb", bufs=2) as cmb:
            for tm in range(TCH):
                g1 = cmb.tile([128, D], BF16, tag="g1")
                nc.gpsimd.indirect_dma_start(
                    out=g1[:], out_offset=None, in_=y_recv[:],
                    in_offset=bass.IndirectOffsetOnAxis(
                        ap=pos_i[:, 2 * tm:2 * tm + 1], axis=0))
                g2 = cmb.tile([128, D], BF16, tag="g2")
                nc.gpsimd.indirect_dma_start(
                    out=g2[:], out_offset=None, in_=y_recv[:],
                    in_offset=bass.IndirectOffsetOnAxis(
                        ap=pos_i[:, 2 * tm + 1:2 * tm + 2], axis=0))
                yc = cmb.tile([128, D], F32, tag="yc")
                t2 = cmb.tile([128, D], F32, tag="t2")
                nc.vector.tensor_scalar_mul(yc[:], g1[:], wsv[:, 2 * tm:2 * tm + 1])
                nc.vector.tensor_scalar_mul(t2[:], g2[:],
                                            wsv[:, 2 * tm + 1:2 * tm + 2])
                nc.vector.tensor_tensor(yc[:], yc[:], t2[:], OP.add)
                for k in range(DK):
                    ptd = ptr.tile([128, 128], F32, tag="ptr")
                    nc.tensor.transpose(ptd[:], yc[:, k * 128:(k + 1) * 128],
                                        ident[:])
                    mo = sbt.tile([128, 128], F32, tag="mo128")
                    nc.vector.tensor_tensor(mo[:], ptd[:],
                                            dsT[:, k, tm * 128:(tm + 1) * 128],
                                            OP.add)
                    nc.vector.tensor_scalar_mul(mo[:], mo[:], 0.5)
                    nc.vector.tensor_tensor(xln2[:, k, tm * 128:(tm + 1) * 128],
                                            mo[:], xT[:, k, tm * 128:(tm + 1) * 128],
                                            OP.add)
        layer_norm_(xT, xln2, lb["ln2w"], lb["ln2b"], 1e-5, bdst=xB)
        dsT_pool.__exit__(None, None, None)

    dbg("xfinal", xT[:])
    # ---------------- final rms + allgather + lm_head
    rmsw = cst.tile([128, DK], F32, name="rmsw_sb")
    nc.sync.dma_start(rmsw[:], P["rmsw"][:])
    LMDT0 = BF16 if c.get("lm_bf16", True) else F32R
    xf_in = drp.tile([128, DK, TC], LMDT0, name="xfin")
    xf_all = drp.tile([NC, 128, DK, TC], LMDT0, name="xfall", addr_space="Shared")
    xr = xlp.tile([128, DK, TC], F32R, tag="xln")
    layer_norm_(xr, xT, rmsw, None, 1e-6, skip_mean=True)
    LMDT = BF16 if c.get("lm_bf16", True) else F32R
    xrb = xlp.tile([128, DK, TC], LMDT, tag="xrb")
    for k in range(DK):
        nc.vector.tensor_copy(xrb[:, k, :], xr[:, k, :])
    nc.sync.dma_start(xf_in[:], xrb[:])
    nc.gpsimd.collective_compute(
        "AllGather", OP.bypass, replica_groups=GRP_ALL,
        ins=[xf_in[:]], outs=[xf_all[:]])
    with (
        tc.tile_pool(name="lph", bufs=2) as lph,
        tc.tile_pool(name="lphw", bufs=8) as lphw,
    ):
        for n in range(NC):
            xfn = lph.tile([128, DK, TC], LMDT0, tag="xan")
            nc.sync.dma_start(xfn[:], xf_all[n])
            for m in range(VCK):
                wt = lphw.tile([128, DK, 128], LMDT0, tag="wt")
                nc.sync.dma_start(wt[:], P["embT"][m])
                ps = psm.tile([128, TC], F32, tag="psmm")
                for k in range(DK):
                    nc.tensor.matmul(ps[:], wt[:, k, :], xfn[:, k, :],
                                     start=(k == 0), stop=(k == DK - 1))
                lo = sbt.tile([128, TC], F32, tag="lo")
                nc.vector.tensor_copy(lo[:], ps[:])
                rows = min(128, VC - m * 128)
                nc.sync.dma_start(
                    OUT[m * 128:m * 128 + rows, n * TC:(n + 1) * TC], lo[:rows, :])

    es.close()


# ---------------------------------------------------------------- runner

def run_model(inputs, cfg, nc=None):
    c = derived(cfg)
    in_maps = prep_in_maps(inputs, cfg)
    if nc is None:
        nc = build_nc(cfg)
    res = run_bass_kernel_spmd(nc, in_maps, core_ids=list(range(c["NC"])))
    return assemble_logits(res.results, cfg), nc


# ---------------------------------------------------------------- entry point

_NC_CACHE = None


def kernel(**inputs):
    """Full-model forward on 8 trn2 cores. inputs as in reference.setup_inputs()."""
    global _NC_CACHE
    import numpy as _np
    inputs = {k: _np.asarray(v) for k, v in inputs.items()}
    if _NC_CACHE is None:
        _NC_CACHE = build_nc(FULL_CFG)
    in_maps = prep_in_maps(inputs, FULL_CFG)
    res = run_bass_kernel_spmd(_NC_CACHE, in_maps,
                               core_ids=list(range(FULL_CFG["NC"])))
    return assemble_logits(res.results, FULL_CFG)



# revision 34
# speedup vs baseline: 1.1053x; 1.0229x over previous
"""Bass/Tile kernel for nn_DeepseekV3MLPMoEModel on 8 trn2 cores.

Sharding: data-parallel over tokens (T/8 per core) for attention/MLP/lm_head
(vocab-sharded), expert-parallel for the MoE (1 expert/core, dense over all
tokens, ReduceScatter of the weighted sum).

Residual stream layout on device: xT [D(part-chunks of 128), T_loc] (f32r).
"""
import sys
sys.path.insert(0, "/opt/trn_rl_repo")
import numpy as np
import concourse.bass as bass
import concourse.mybir as mybir
import concourse.tile as tile
from concourse import bacc
from concourse.bass_utils import run_bass_kernel_spmd
from concourse.masks import make_identity

F32 = mybir.dt.float32
BF16 = mybir.dt.bfloat16
F32R = mybir.dt.float32r
I32 = mybir.dt.int32
AF = mybir.ActivationFunctionType
OP = mybir.AluOpType
AX = mybir.AxisListType

FULL_CFG = dict(B=2, S=2048, D=1024, H=16, F=2048, E=8, V=32000, L=2, NC=8, G=4,
                C2=192)
MINI_CFG = dict(B=2, S=512, D=256, H=4, F=512, E=8, V=1024, L=2, NC=8, G=4,
                C2=64)


def derived(cfg):
    c = dict(cfg)
    c["T"] = c["B"] * c["S"]
    c["TC"] = c["T"] // c["NC"]          # tokens per core
    c["TCH"] = c["TC"] // 128            # token tiles per core
    c["DK"] = c["D"] // 128              # D chunks
    c["FK"] = c["F"] // 128              # F chunks
    c["VC"] = c["V"] // c["NC"]          # vocab per core
    c["VCP"] = ((c["VC"] + 127) // 128) * 128
    c["VCK"] = c["VCP"] // 128
    c["VS"] = c["D"] // c["TC"]          # v slots per token-tile in kv pack
    c["SLOTS"] = c["DK"] + c["TCH"] * c["VS"]
    c["dh"] = c["D"] // c["H"]
    assert c["dh"] == 64
    return c


# ---------------------------------------------------------------- host prep

def lhsT_tiles(W, bf16=True):
    """W [M, K] (for out = x @ W.T) -> [M/128, 128(ki), K/128(ko), 128(mm)]."""
    import ml_dtypes
    M, K = W.shape
    Wt = np.ascontiguousarray(W.T)
    r = np.ascontiguousarray(
        Wt.reshape(K // 128, 128, M // 128, 128).transpose(2, 1, 0, 3))
    return r.astype(ml_dtypes.bfloat16) if bf16 else r


def rhs_tiles(W, bf16=False):
    """W [N, K] (used as rhs [K, N]) -> [K/128, 128, N]."""
    import ml_dtypes
    N, K = W.shape
    r = np.ascontiguousarray(W.T.reshape(K // 128, 128, N))
    return r.astype(ml_dtypes.bfloat16) if bf16 else r


def pp_cols(b):
    """b [M] -> [128, M/128]: column m holds b[m*128:(m+1)*128]."""
    return np.ascontiguousarray(b.reshape(-1, 128).T)


def prep_in_maps(inputs, cfg):
    c = derived(cfg)
    NC, L, D, E = c["NC"], c["L"], c["D"], c["E"]
    VC, VCP = c["VC"], c["VCP"]
    f32 = np.float32

    tokens = np.asarray(inputs["tokens"]).astype(np.int64).reshape(-1)  # [T]
    emb = np.asarray(inputs["emb"], f32)

    shared = {}
    for l in range(L):
        ipw = np.asarray(inputs["in_proj_w"][l], f32)     # [3D, D]
        ipb = np.asarray(inputs["in_proj_b"][l], f32)     # [3D]
        bqk = ipb[:2 * D].copy()
        bqk[:D] *= 0.125
        shared[f"wqk{l}"] = lhsT_tiles(ipw[:2 * D], bf16=True)
        shared[f"bqk{l}"] = pp_cols(bqk)
        shared[f"wv{l}"] = rhs_tiles(ipw[2 * D:], bf16=True)
        shared[f"bv{l}"] = ipb[2 * D:].reshape(1, D).copy()
        shared[f"wo{l}"] = lhsT_tiles(np.asarray(inputs["out_proj_w"][l], f32), bf16=True)
        shared[f"bo{l}"] = pp_cols(np.asarray(inputs["out_proj_b"][l], f32))
        for nm in ("ln1_w", "ln1_b", "ln2_w", "ln2_b"):
            shared[f"{nm.replace('_','')}{l}"] = pp_cols(np.asarray(inputs[nm][l], f32))
        shared[f"wg{l}"] = lhsT_tiles(np.asarray(inputs["ds_gate_w"][l], f32), bf16=True)
        shared[f"wu{l}"] = lhsT_tiles(np.asarray(inputs["ds_up_w"][l], f32), bf16=True)
        shared[f"wd{l}"] = lhsT_tiles(np.asarray(inputs["ds_down_w"][l], f32), bf16=True)
        shared[f"gw{l}"] = rhs_tiles(np.asarray(inputs["gate_w"][l], f32))
        shared[f"gb{l}"] = np.asarray(inputs["gate_b"][l], f32).reshape(1, E).copy()
    shared["rmsw"] = pp_cols(np.asarray(inputs["rms_w"], f32))
    shared["ones_mat"] = np.ones((128, 128), f32)
    import ml_dtypes
    shared["ones_bf"] = np.ones((128, 64), ml_dtypes.bfloat16)
    shared["triu"] = np.triu(np.ones((128, 128), f32), 1)
    shared["ebase"] = (np.arange(E) * 64).astype(f32).reshape(1, E)

    in_maps = []
    for core in range(NC):
        m = dict(shared)
        lo = core * VC
        m["embrows"] = emb  # replicated full table
        loc = tokens[core * (len(tokens) // NC):(core + 1) * (len(tokens) // NC)]
        m["tokidx"] = np.ascontiguousarray(
            loc.reshape(-1, 128).T.astype(np.int32))  # [128, TC/128]
        esl = np.zeros((VCP, D), f32)
        esl[:VC] = emb[lo:lo + VC]
        m["embT"] = lhsT_tiles(esl, bf16=c.get("lm_bf16", True))
        for l in range(L):
            m[f"w1{l}"] = lhsT_tiles(np.asarray(inputs["moe_w1"][l, core], f32), bf16=True)
            m[f"b1{l}"] = pp_cols(np.asarray(inputs["moe_b1"][l, core], f32))
            m[f"w2{l}"] = lhsT_tiles(np.asarray(inputs["moe_w2"][l, core], f32), bf16=True)
            m[f"b2{l}"] = pp_cols(np.asarray(inputs["moe_b2"][l, core], f32))
        in_maps.append(m)
    return in_maps


def assemble_logits(results, cfg):
    c = derived(cfg)
    B, S, V, VC = c["B"], c["S"], c["V"], c["VC"]
    out = np.empty((B, S, V), np.float32)
    for core, r in enumerate(results):
        lg = r["logits"]  # [VC, T]
        out[:, :, core * VC:(core + 1) * VC] = lg.T.reshape(B, S, VC)
    return out


# ---------------------------------------------------------------- device code

def build_nc(cfg):
    c = derived(cfg)
    L, D, E = c["L"], c["D"], c["E"]
    DK, FK = c["DK"], c["FK"]
    VC, VCK = c["VC"], c["VCK"]
    T = c["T"]

    nc = bacc.Bacc(None)
    P = {}

    def par(name, shape, dt):
        P[name] = nc.dram_tensor(name, shape, dt, kind="ExternalInput")

    par("tokidx", [128, T // (8 * 128)], I32)
    par("ones_mat", [128, 128], F32R)
    par("ones_bf", [128, 64], BF16)
    par("triu", [128, 128], F32R)
    par("ebase", [1, E], F32R)
    par("embrows", [c["V"], D], F32)
    par("embT", [VCK, 128, DK, 128], BF16 if c.get("lm_bf16", True) else F32R)
    for l in range(L):
        par(f"wqk{l}", [2 * DK, 128, DK, 128], BF16)
        par(f"bqk{l}", [128, 2 * DK], F32)
        par(f"wv{l}", [DK, 128, D], BF16)
        par(f"bv{l}", [1, D], F32R)
        par(f"wo{l}", [DK, 128, DK, 128], BF16)
        par(f"bo{l}", [128, DK], F32)
        for nm in ("ln1w", "ln1b", "ln2w", "ln2b"):
            par(f"{nm}{l}", [128, DK], F32)
        par(f"wg{l}", [FK, 128, DK, 128], BF16)
        par(f"wu{l}", [FK, 128, DK, 128], BF16)
        par(f"wd{l}", [DK, 128, FK, 128], BF16)
        par(f"gw{l}", [DK, 128, E], F32)
        par(f"gb{l}", [1, E], F32R)
        par(f"w1{l}", [FK, 128, DK, 128], BF16)
        par(f"b1{l}", [128, FK], F32)
        par(f"w2{l}", [DK, 128, FK, 128], BF16)
        par(f"b2{l}", [128, DK], F32)
    par("rmsw", [128, DK], F32)
    OUT = nc.dram_tensor("logits", [VC, T], F32, kind="ExternalOutput")

    with tile.TileContext(nc) as tc:
        _emit(nc, tc, P, OUT, c)
    nc.compile()
    return nc


def _emit(nc, tc, P, OUT, c):
    NC, L, D, H, F, E = c["NC"], c["L"], c["D"], c["H"], c["F"], c["E"]
    TC, TCH, DK, FK = c["TC"], c["TCH"], c["DK"], c["FK"]
    VC, VCK, VS, SLOTS = c["VC"], c["VCK"], c["VS"], c["SLOTS"]
    G, T = c["G"], c["T"]
    KCH = G * TCH
    TK = T // 128
    NDN = max(1, D // 512)
    NW = min(512, D)
    GRP_KV = [list(range(g * G, (g + 1) * G)) for g in range(NC // G)]
    GRP_ALL = [list(range(NC))]

    from contextlib import ExitStack
    es = ExitStack()
    cst = es.enter_context(tc.tile_pool(name="cst", bufs=1))
    sbt = es.enter_context(tc.tile_pool(name="sbt", bufs=2))
    lnp = es.enter_context(tc.tile_pool(name="lnp", bufs=2))
    xlp = es.enter_context(tc.tile_pool(name="xlp", bufs=1))
    psm = es.enter_context(tc.tile_pool(name="psm", bufs=3, space="PSUM"))
    pst = es.enter_context(tc.tile_pool(name="pst", bufs=2, space="PSUM"))
    ptr = es.enter_context(tc.tile_pool(name="ptr", bufs=1, space="PSUM"))
    drp = es.enter_context(tc.tile_pool(name="drp", bufs=1, space="DRAM"))

    dbg_on = c.get("debug", False)

    def dbg(name, ap):
        if not dbg_on:
            return
        t = nc.dram_tensor(f"dbg_{name}", list(ap.shape), ap.dtype,
                           kind="ExternalOutput")
        nc.sync.dma_start(t[:], ap)

    ident = cst.tile([128, 128], F32, name="ident")
    make_identity(nc, ident)
    identB = cst.tile([128, 128], BF16, name="identB")
    nc.vector.tensor_copy(identB[:], ident[:])
    ones_m = cst.tile([128, 128], F32R, name="ones_m")
    nc.sync.dma_start(ones_m[:], P["ones_mat"][:])
    triu_sb = cst.tile([128, 128], F32R, name="triu_sb")
    nc.sync.dma_start(triu_sb[:], P["triu"][:])
    ebase_sb = cst.tile([1, E], F32R, name="ebase_sb")
    nc.sync.dma_start(ebase_sb[:], P["ebase"][:])
    ptb = es.enter_context(tc.tile_pool(name="ptb", bufs=2, space="PSUM"))
    eps5 = cst.tile([128, 1], F32, name="eps5")
    nc.gpsimd.memset(eps5[:], 1e-5)
    eps6 = cst.tile([128, 1], F32, name="eps6")
    nc.gpsimd.memset(eps6[:], 1e-6)
    xT = cst.tile([128, DK, TC], F32R, name="xT")
    xB = cst.tile([128, DK, TC], BF16, name="xB")
    
    KCH_ = G * TCH


    # ---------------- embedding: gather own tokens from replicated table
    with tc.tile_pool(name="emb_ph", bufs=3) as ph:
        idx_sb = ph.tile([128, TCH], I32, name="idx_sb", bufs=1)
        nc.sync.dma_start(idx_sb[:], P["tokidx"][:])
        sqrt_d = float(np.sqrt(c["D"]))
        for tm in range(TCH):
            ge = ph.tile([128, D], F32, tag="ge")
            nc.gpsimd.indirect_dma_start(
                out=ge[:], out_offset=None, in_=P["embrows"][:],
                in_offset=bass.IndirectOffsetOnAxis(ap=idx_sb[:, tm:tm + 1], axis=0))
            for k in range(DK):
                pt = ptr.tile([128, 128], F32, tag="ptr")
                nc.tensor.transpose(pt[:], ge[:, k * 128:(k + 1) * 128], ident[:])
                nc.scalar.activation(xT[:, k, tm * 128:(tm + 1) * 128], pt[:],
                                     AF.Copy, scale=sqrt_d)
                nc.vector.tensor_copy(xB[:, k, tm * 128:(tm + 1) * 128],
                                      xT[:, k, tm * 128:(tm + 1) * 128])
    dbg("x0T", xT[:])

    # ---------------- LN helper (matmul stats, replicated across partitions)
    def layer_norm_(dst, src, wcols, bcols, eps, skip_mean=False, bdst=None):
        eps = eps5[:, 0:1] if eps == 1e-5 else eps6[:, 0:1]
        ps1 = None if skip_mean else pst.tile([128, TC], F32, tag="pstat")
        ps2 = pst.tile([128, TC], F32, tag="pstat")
        for k in range(DK):
            sq = lnp.tile([128, TC], F32R, tag="sq")
            nc.vector.tensor_tensor(sq[:], src[:, k, :], src[:, k, :], OP.mult)
            if not skip_mean:
                nc.tensor.matmul(ps1[:], ones_m[:], src[:, k, :],
                                 start=(k == 0), stop=(k == DK - 1))
            nc.tensor.matmul(ps2[:], ones_m[:], sq[:],
                             start=(k == 0), stop=(k == DK - 1))
        e2 = lnp.tile([128, TC], F32, tag="stmp")
        nc.scalar.activation(e2[:], ps2[:], AF.Copy, scale=1.0 / c["D"])
        if not skip_mean:
            mu = lnp.tile([128, TC], F32, tag="smu", bufs=1)
            nc.scalar.activation(mu[:], ps1[:], AF.Copy, scale=1.0 / c["D"])
            var = lnp.tile([128, TC], F32, tag="stmp")
            nc.vector.tensor_tensor(var[:], mu[:], mu[:], OP.mult)
            nc.vector.tensor_tensor(var[:], e2[:], var[:], OP.subtract)
        else:
            var = e2
        sd = lnp.tile([128, TC], F32, tag="stmp")
        nc.scalar.activation(sd[:], var[:], AF.Sqrt, bias=eps)
        rstd = lnp.tile([128, TC], F32, tag="srstd", bufs=1)
        nc.vector.reciprocal(rstd[:], sd[:])
        for k in range(DK):
            t1 = lnp.tile([128, TC], F32, tag="lnt")
            if not skip_mean:
                nc.vector.tensor_tensor(t1[:], src[:, k, :], mu[:], OP.subtract)
                nc.vector.tensor_tensor(t1[:], t1[:], rstd[:], OP.mult)
            else:
                nc.vector.tensor_tensor(t1[:], src[:, k, :], rstd[:], OP.mult)
            if bcols is not None:
                nc.vector.tensor_scalar(dst[:, k, :], t1[:],
                                        wcols[:, k:k + 1], bcols[:, k:k + 1],
                                        OP.mult, OP.add)
            else:
                nc.vector.tensor_scalar_mul(dst[:, k, :], t1[:], wcols[:, k:k + 1])
            if bdst is not None:
                nc.scalar.activation(bdst[:, k, :], dst[:, k, :], AF.Copy)

    # ---------------- layers
    for l in range(L):
        lb = {}
        for nm in ("bqk", "bo", "ln1w", "ln1b", "ln2w", "ln2b", "b1", "b2"):
            w = P[f"{nm}{l}"].shape[1]
            t = cst.tile([128, w], F32, name=f"{nm}{l}_sb", tag=f"c_{nm}")
            nc.sync.dma_start(t[:], P[f"{nm}{l}"][:])
            lb[nm] = t
        bv1 = cst.tile([1, D], F32R, name=f"bv1_{l}", tag="c_bv1")
        nc.sync.dma_start(bv1[:], P[f"bv{l}"][:])
        bv = cst.tile([128, D], F32, name=f"bv{l}_sb", tag="c_bv")
        for dn in range(NDN):
            psb = psm.tile([128, NW], F32, tag="psmm")
            nc.tensor.matmul(psb[:], ones_m[0:1, :],
                             bv1[0:1, dn * NW:(dn + 1) * NW], start=True, stop=True)
            nc.vector.tensor_copy(bv[:, dn * NW:(dn + 1) * NW], psb[:])
        gb1 = cst.tile([1, E], F32R, name=f"gb1_{l}", tag="c_gb1")
        nc.sync.dma_start(gb1[:], P[f"gb{l}"][:])
        psgb = psm.tile([128, E], F32, tag="psmm")
        nc.tensor.matmul(psgb[:], ones_m[0:1, :], gb1[0:1, :], start=True, stop=True)
        gb = cst.tile([128, E], F32, name=f"gb{l}_sb", tag="c_gb")
        nc.vector.tensor_copy(gb[:], psgb[:])

        assert NDN == 2 and NW == TC and VS == 2
        kv_ink = [drp.tile([128, TC], BF16, name=f"kvink{m}", tag=f"kvink{m}")
                  for m in range(DK)]
        kv_allk = [drp.tile([G, 128, TC], BF16, name=f"kvallk{m}",
                            tag=f"kvallk{m}") for m in range(DK)]
        kv_inv = [drp.tile([TCH, 128, TC], BF16, name=f"kvinv{dn}",
                           tag=f"kvinv{dn}") for dn in range(NDN)]
        kv_allv = [drp.tile([G, TCH, 128, TC], BF16, name=f"kvallv{dn}",
                            tag=f"kvallv{dn}") for dn in range(NDN)]

        # --- qkv phase: k first (per-slot gathers pipeline), then q, then v
        with tc.tile_pool(name="qp", bufs=1) as qp:
            q_sb = qp.tile([128, DK, TC], BF16, tag="q_sb")
            with (
                tc.tile_pool(name="qphw", bufs=4) as qphw,
                tc.tile_pool(name="qphk", bufs=2) as qphk,
                tc.tile_pool(name="qpv", bufs=1) as qpv,
            ):
                wv0 = qpv.tile([128, DK, NW], BF16, tag="wv0")
                wv1 = qpv.tile([128, DK, NW], BF16, tag="wv1")
                for dn, wv in enumerate((wv0, wv1)):
                    for k in range(DK):
                        nc.sync.dma_start(wv[:, k, :],
                                          P[f"wv{l}"][k, :, dn * NW:(dn + 1) * NW])

                def v_pass(dn, wv):
                    for tm in range(TCH):
                        ps = psm.tile([128, NW], F32, tag="psmm")
                        for k in range(DK):
                            nc.tensor.matmul(ps[:], xB[:, k, tm * 128:(tm + 1) * 128],
                                             wv[:, k, :],
                                             start=(k == 0), stop=(k == DK - 1))
                        vt = qphk.tile([128, NW], BF16, tag="vt")
                        nc.vector.tensor_tensor(
                            vt[:], ps[:], bv[:, dn * NW:(dn + 1) * NW], OP.add)
                        nc.sync.dma_start(kv_inv[dn][tm], vt[:])
                    nc.gpsimd.collective_compute(
                        "AllGather", OP.bypass, replica_groups=GRP_KV,
                        ins=[kv_inv[dn][:]], outs=[kv_allv[dn][:]])

                v_pass(0, wv0)                       # v0 transfers first
                for m in range(2 * DK):
                    mm = (m + DK) % (2 * DK)        # k chunks before q
                    wt = qphw.tile([128, DK, 128], BF16, tag="wt")
                    nc.sync.dma_start(wt[:], P[f"wqk{l}"][mm])
                    ps = psm.tile([128, TC], F32, tag="psmm")
                    for k in range(DK):
                        nc.tensor.matmul(ps[:], wt[:, k, :], xB[:, k, :],
                                         start=(k == 0), stop=(k == DK - 1))
                    if mm < DK:
                        nc.scalar.activation(q_sb[:, mm, :], ps[:], AF.Identity,
                                             scale=0.125, bias=lb["bqk"][:, mm:mm + 1])
                    else:
                        kt = qphk.tile([128, TC], BF16, tag="kt")
                        nc.scalar.activation(kt[:], ps[:], AF.Identity,
                                             bias=lb["bqk"][:, mm:mm + 1])
                        nc.sync.dma_start(kv_ink[mm - DK][:], kt[:])
                        nc.gpsimd.collective_compute(
                            "AllGather", OP.bypass, replica_groups=GRP_KV,
                            ins=[kv_ink[mm - DK][:]], outs=[kv_allk[mm - DK][:]])
                v_pass(1, wv1)
            if l == 0:
                dbg("q0", q_sb[:])

            # --- attention (q_sb in scope)
            with tc.tile_pool(name="aoT", bufs=1) as aoTp:
                oT = aoTp.tile([128, DK, TC], BF16, tag="oT")
                vh2 = aoTp.tile([128, 2, KCH, 128], BF16, tag="vh2")
                for b_ in range(2):
                    for kc_ in range(KCH):
                        nc.sync.dma_start(vh2[:, b_, kc_, 64:128],
                                          P["ones_bf"][:, 0:64])
                with (
                    tc.tile_pool(name="aph", bufs=2) as aph,
                    tc.tile_pool(name="apT", bufs=2) as apTp,
                ):
                    for h in range(H):
                        qm, qoff = h // 2, 64 * (h % 2)
                        kh = aph.tile([128, G, TC], BF16, tag="kh")
                        for g in range(G):
                            nc.sync.dma_start(kh[qoff:qoff + 64, g, :],
                                              kv_allk[qm][g, qoff:qoff + 64, :])
                        s_v, off_v = h // (TC // 64), (64 * h) % TC
                        for g in range(G):
                            for tm in range(TCH):
                                nc.sync.dma_start(
                                    vh2[:, h % 2, g * TCH + tm, 0:64],
                                    kv_allv[s_v][g, tm, :, off_v:off_v + 64])
                        pT = apTp.tile([128, KCH, TC], BF16, tag="pT")
                        for kc in range(KCH):
                            ps = psm.tile([128, TC], F32, tag="psmm")
                            nc.tensor.matmul(
                                ps[:],
                                kh[qoff:qoff + 64, kc // TCH,
                                   (kc % TCH) * 128:(kc % TCH) * 128 + 128],
                                q_sb[qoff:qoff + 64, qm, :], start=True, stop=True)
                            nc.scalar.activation(pT[:, kc, :], ps[:], AF.Exp)
                        po = psm.tile([128, TC], F32, tag="psmm")
                        for kc in range(KCH):
                            nc.tensor.matmul(po[:], vh2[:, h % 2, kc, :],
                                             pT[:, kc, :],
                                             start=(kc == 0), stop=(kc == KCH - 1))
                        rec = sbt.tile([64, TC], F32, tag="rec")
                        nc.vector.reciprocal(rec[:], po[64:128, :])
                        nc.vector.tensor_tensor(oT[qoff:qoff + 64, qm, :],
                                                po[0:64, :], rec[:], OP.mult)
                if l == 0:
                    dbg("oT0", oT[:])
                # --- out proj + residual + ln1
                with tc.tile_pool(name="oph", bufs=4) as oph:
                    xln = xlp.tile([128, DK, TC], F32R, tag="xln")
                    for m in range(DK):
                        wt = oph.tile([128, DK, 128], BF16, tag="wt")
                        nc.sync.dma_start(wt[:], P[f"wo{l}"][m])
                        ps = psm.tile([128, TC], F32, tag="psmm")
                        for k in range(DK):
                            nc.tensor.matmul(ps[:], wt[:, k, :], oT[:, k, :],
                                             start=(k == 0), stop=(k == DK - 1))
                        t = sbt.tile([128, TC], F32, tag="ot")
                        nc.vector.tensor_scalar_add(t[:], ps[:], lb["bo"][:, m:m + 1])
                        nc.vector.tensor_tensor(xln[:, m, :], t[:], xT[:, m, :],
                                                OP.add)
                    layer_norm_(xT, xln, lb["ln1w"], lb["ln1b"], 1e-5, bdst=xB)
        if l == 0:
            dbg("xln1_0", xT[:])

        # --- router: gate scores -> top2 masks -> capacity slots -> x scatter
        C2 = c["C2"]
        CE = E * C2
        NCH = CE // TC
        assert NCH * TC == CE
        assert C2 == 192 and CE == 3 * TC
        x_send = drp.tile([CE, D], BF16, name="xsend", tag="xsend")
        x_recv = [drp.tile([TC, D], BF16, name=f"xrecv{cc}", tag=f"xrecv{cc}")
                  for cc in range(NCH)]
        y_send = [drp.tile([TC, D], BF16, name=f"ysend{cc}", tag=f"ysend{cc}")
                  for cc in range(NCH)]
        y_recv = drp.tile([CE, D], BF16, name="yrecv", tag="yrecv")
        pos_i = cst.tile([128, 2 * TCH], I32, name=f"posi{l}", tag="c_posi")
        wsv = cst.tile([128, 2 * TCH], F32, name=f"wsv{l}", tag="c_wsv")
        with tc.tile_pool(name="rph", bufs=2) as rph:
            gwt = rph.tile([128, DK, E], F32, tag="gwt", bufs=1)
            for k in range(DK):
                nc.sync.dma_start(gwt[:, k, :], P[f"gw{l}"][k])
            base_row = rph.tile([1, E], F32R, tag="base", bufs=1)
            nc.vector.tensor_scalar_mul(base_row[:], ebase_sb[:], 0.0)
            pseb = psm.tile([128, E], F32, tag="psmm")
            nc.tensor.matmul(pseb[:], ones_m[0:1, :], ebase_sb[0:1, :],
                             start=True, stop=True)
            e64b = rph.tile([128, E], F32, tag="e64b", bufs=1)
            nc.vector.tensor_copy(e64b[:], pseb[:])
            for tm in range(TCH):
                xf = rph.tile([128, DK, 128], F32, tag="xf")
                for k in range(DK):
                    nc.vector.tensor_copy(xf[:, k, :],
                                          xT[:, k, tm * 128:(tm + 1) * 128])
                psg = psm.tile([128, E], F32, tag="psmm")
                for k in range(DK):
                    nc.tensor.matmul(psg[:], xf[:, k, :], gwt[:, k, :],
                                     start=(k == 0), stop=(k == DK - 1))
                gs = rph.tile([128, E], F32, tag="gs")
                nc.vector.tensor_tensor(gs[:], psg[:], gb[:], OP.add)
                m1 = rph.tile([128, 1], F32, tag="m1")
                nc.vector.tensor_reduce(m1[:], gs[:], AX.X, OP.max)
                mask1 = rph.tile([128, E], F32, tag="mask1")
                nc.vector.tensor_tensor(mask1[:], gs[:],
                                        m1[:].to_broadcast([128, E]), OP.is_equal)
                gs2 = rph.tile([128, E], F32, tag="gs2")
                nc.vector.tensor_scalar_mul(gs2[:], mask1[:], -1e30)
                nc.vector.tensor_tensor(gs2[:], gs2[:], gs[:], OP.add)
                m2 = rph.tile([128, 1], F32, tag="m2")
                nc.vector.tensor_reduce(m2[:], gs2[:], AX.X, OP.max)
                mask2 = rph.tile([128, E], F32, tag="mask2")
                nc.vector.tensor_tensor(mask2[:], gs2[:],
                                        m2[:].to_broadcast([128, E]), OP.is_equal)
                dm = rph.tile([128, 1], F32, tag="dm")
                nc.vector.tensor_tensor(dm[:], m2[:], m1[:], OP.subtract)
                nc.scalar.activation(dm[:], dm[:], AF.Exp)
                nc.vector.tensor_scalar_add(dm[:], dm[:], 1.0)
                w1t = rph.tile([128, 1], F32, tag="w1t")
                nc.vector.reciprocal(w1t[:], dm[:])
                nc.vector.tensor_copy(wsv[:, 2 * tm:2 * tm + 1], w1t[:])
                nc.vector.tensor_scalar(wsv[:, 2 * tm + 1:2 * tm + 2], w1t[:],
                                        -1.0, 1.0, OP.mult, OP.add)
                # combined mask -> exclusive prefix rank per expert
                me = rph.tile([128, E], F32R, tag="me")
                nc.vector.tensor_tensor(me[:], mask1[:], mask2[:], OP.add)
                pse = psm.tile([128, E], F32, tag="psmm")
                nc.tensor.matmul(pse[:], triu_sb[:], me[:], start=True, stop=True)
                psb = psm.tile([128, E], F32, tag="psmm")
                nc.tensor.matmul(psb[:], ones_m[0:1, :], base_row[0:1, :],
                                 start=True, stop=True)
                bb = rph.tile([128, E], F32, tag="bb")
                nc.vector.tensor_copy(bb[:], psb[:])
                rankg = rph.tile([128, E], F32, tag="rankg")
                nc.vector.tensor_tensor(rankg[:], pse[:], bb[:], OP.add)
                nc.vector.tensor_scalar_min(rankg[:], rankg[:], float(C2 - 1))
                # chunk id c = (r>63) + (r>127) via clamp(relu(r-k),0,1)
                c1t = rph.tile([128, E], F32, tag="c1t")
                nc.vector.tensor_scalar(c1t[:], rankg[:], -63.0, 0.0,
                                        OP.add, OP.max)
                nc.vector.tensor_scalar_min(c1t[:], c1t[:], 1.0)
                c2t = rph.tile([128, E], F32, tag="c2t")
                nc.vector.tensor_scalar(c2t[:], rankg[:], -127.0, 0.0,
                                        OP.add, OP.max)
                nc.vector.tensor_scalar_min(c2t[:], c2t[:], 1.0)
                nc.vector.tensor_tensor(c1t[:], c1t[:], c2t[:], OP.add)
                # slot = r + (TC-64)*c + 64*e
                slotf = rph.tile([128, E], F32, tag="slotf")
                nc.vector.tensor_scalar(slotf[:], c1t[:], float(TC - 64), None,
                                        OP.mult)
                nc.vector.tensor_tensor(slotf[:], slotf[:], rankg[:], OP.add)
                nc.vector.tensor_tensor(slotf[:], slotf[:], e64b[:], OP.add)
                pstt = psm.tile([1, E], F32, tag="psmm")
                nc.tensor.matmul(pstt[:], ones_m[:, 0:1], me[:],
                                 start=True, stop=True)
                nc.vector.tensor_tensor(base_row[:], base_row[:], pstt[0:1, :],
                                        OP.add)
                for j, msk in ((0, mask1), (1, mask2)):
                    tt = rph.tile([128, E], F32, tag="tt")
                    nc.vector.tensor_tensor(tt[:], msk[:], slotf[:], OP.mult)
                    posf = rph.tile([128, 1], F32, tag="posf")
                    nc.vector.tensor_reduce(posf[:], tt[:], AX.X, OP.add)
                    nc.vector.tensor_copy(pos_i[:, 2 * tm + j:2 * tm + j + 1],
                                          posf[:])
                xrow = rph.tile([128, D], BF16, tag="xrow")
                for kk in range(DK // 4):
                    ptb_ = ptb.tile([128, 4, 128], BF16, tag="ptb")
                    for k4 in range(4):
                        nc.tensor.transpose(
                            ptb_[:, k4, :],
                            xB[:, kk * 4 + k4, tm * 128:(tm + 1) * 128], identB[:])
                    nc.scalar.activation(xrow[:, kk * 512:(kk + 1) * 512], ptb_[:],
                                         AF.Copy)
                for j in range(2):
                    nc.gpsimd.indirect_dma_start(
                        out=x_send[:],
                        out_offset=bass.IndirectOffsetOnAxis(
                            ap=pos_i[:, 2 * tm + j:2 * tm + j + 1], axis=0),
                        in_=xrow[:], in_offset=None)
        if l == 0 and dbg_on:
            dbg("pos0", pos_i[:])
            dbg("wsv0", wsv[:])
            with tc.tile_pool(name="dbgp", bufs=1) as dbgp:
                xs0 = dbgp.tile([128, D], BF16, tag="xs0")
                nc.sync.dma_start(xs0[:], x_send[0:128, :])
                dbg("xsend00", xs0[:])
        for cc in range(NCH):
            nc.gpsimd.collective_compute(
                "AllToAll", OP.bypass, replica_groups=GRP_ALL,
                ins=[x_send[cc * TC:(cc + 1) * TC, :]], outs=[x_recv[cc][:]])

        # --- ds mlp (local tokens; hides the x AllToAll)
        dsT_pool = tc.tile_pool(name="dsT", bufs=1)
        dsTp = dsT_pool.__enter__()
        dsT = dsTp.tile([128, DK, TC], F32, tag="dsT")
        with (
            tc.tile_pool(name="dph", bufs=1) as dph,
            tc.tile_pool(name="dphw", bufs=3) as dphw,
        ):
            gu = dph.tile([128, FK, TC], BF16, tag="gu")
            for m in range(FK):
                wtg = dphw.tile([128, DK, 128], BF16, tag="wt")
                nc.sync.dma_start(wtg[:], P[f"wg{l}"][m])
                psg = psm.tile([128, TC], F32, tag="psmm")
                for k in range(DK):
                    nc.tensor.matmul(psg[:], wtg[:, k, :], xB[:, k, :],
                                     start=(k == 0), stop=(k == DK - 1))
                sg = sbt.tile([128, TC], F32, tag="sg")
                nc.scalar.activation(sg[:], psg[:], AF.Sigmoid)
                nc.vector.tensor_tensor(sg[:], sg[:], psg[:], OP.mult)
                wtu = dphw.tile([128, DK, 128], BF16, tag="wt")
                nc.sync.dma_start(wtu[:], P[f"wu{l}"][m])
                psu = psm.tile([128, TC], F32, tag="psmm")
                for k in range(DK):
                    nc.tensor.matmul(psu[:], wtu[:, k, :], xB[:, k, :],
                                     start=(k == 0), stop=(k == DK - 1))
                nc.vector.tensor_tensor(gu[:, m, :], sg[:], psu[:], OP.mult)
            for m in range(DK):
                wtd = dphw.tile([128, FK, 128], BF16, tag="wtd", bufs=2)
                nc.sync.dma_start(wtd[:], P[f"wd{l}"][m])
                psd = psm.tile([128, TC], F32, tag="psmm")
                for k in range(FK):
                    nc.tensor.matmul(psd[:], wtd[:, k, :], gu[:, k, :],
                                     start=(k == 0), stop=(k == FK - 1))
                nc.vector.tensor_copy(dsT[:, m, :], psd[:])
        if l == 0:
            dbg("dsT0", dsT[:])

        # --- expert pass over routed tokens only, chunk-pipelined
        with (
            tc.tile_pool(name="mph", bufs=2) as mph,
            tc.tile_pool(name="mphh", bufs=1) as mphh,
            tc.tile_pool(name="mphw", bufs=3) as mphw,
            tc.tile_pool(name="mphr", bufs=3) as mphr,
        ):
            for ch in range(NCH):
                co = ch * TC
                xeT = mph.tile([128, DK, TC], BF16, tag="xeT")
                for rt in range(TC // 128):
                    xrt = mphr.tile([128, D], BF16, tag="xrt")
                    nc.sync.dma_start(xrt[:],
                                      x_recv[ch][rt * 128:(rt + 1) * 128, :])
                    for kk in range(DK // 4):
                        ptb_ = ptb.tile([128, 4, 128], BF16, tag="ptb")
                        for k4 in range(4):
                            k = kk * 4 + k4
                            nc.tensor.transpose(ptb_[:, k4, :],
                                                xrt[:, k * 128:(k + 1) * 128],
                                                identB[:])
                        nc.scalar.activation(
                            xeT[:, kk * 4:(kk + 1) * 4, rt * 128:(rt + 1) * 128],
                            ptb_[:], AF.Copy)
                hTc = mphh.tile([128, FK, TC], BF16, tag="hTc")
                for m in range(FK):
                    wt1 = mphw.tile([128, DK, 128], BF16, tag="wt")
                    nc.sync.dma_start(wt1[:], P[f"w1{l}"][m])
                    ps = psm.tile([128, TC], F32, tag="psmm")
                    for k in range(DK):
                        nc.tensor.matmul(ps[:], wt1[:, k, :], xeT[:, k, :],
                                         start=(k == 0), stop=(k == DK - 1))
                    nc.scalar.activation(hTc[:, m, :], ps[:], AF.Relu,
                                         bias=lb["b1"][:, m:m + 1])
                yTc = mphh.tile([128, DK, TC], BF16, tag="yTc")
                for m in range(DK):
                    wt2 = mphw.tile([128, FK, 128], BF16, tag="wtd", bufs=2)
                    nc.sync.dma_start(wt2[:], P[f"w2{l}"][m])
                    ps = psm.tile([128, TC], F32, tag="psmm")
                    for k in range(FK):
                        nc.tensor.matmul(ps[:], wt2[:, k, :], hTc[:, k, :],
                                         start=(k == 0), stop=(k == FK - 1))
                    nc.vector.tensor_scalar_add(yTc[:, m, :], ps[:],
                                                lb["b2"][:, m:m + 1])
                for rt in range(TC // 128):
                    yrt = mphr.tile([128, D], BF16, tag="yrt")
                    for kk in range(DK // 4):
                        ptb_ = ptb.tile([128, 4, 128], BF16, tag="ptb")
                        for k4 in range(4):
                            k = kk * 4 + k4
                            nc.tensor.transpose(ptb_[:, k4, :],
                                                yTc[:, k, rt * 128:(rt + 1) * 128],
                                                identB[:])
                        nc.scalar.activation(yrt[:, kk * 512:(kk + 1) * 512],
                                             ptb_[:], AF.Copy)
                    nc.sync.dma_start(
                        y_send[ch][rt * 128:(rt + 1) * 128, :], yrt[:])
                nc.gpsimd.collective_compute(
                    "AllToAll", OP.bypass, replica_groups=GRP_ALL,
                    ins=[y_send[ch][:]],
                    outs=[y_recv[co:co + TC, :]])

        # --- combine + ln2 (gather own tokens' two expert rows)
        xln2 = xlp.tile([128, DK, TC], F32R, tag="xln")
        with tc.tile_pool(name="cmb", bufs=2) as cmb:
            for tm in range(TCH):
                g1 = cmb.tile([128, D], BF16, tag="g1")
                nc.gpsimd.indirect_dma_start(
                    out=g1[:], out_offset=None, in_=y_recv[:],
                    in_offset=bass.IndirectOffsetOnAxis(
                        ap=pos_i[:, 2 * tm:2 * tm + 1], axis=0))
                g2 = cmb.tile([128, D], BF16, tag="g2")
                nc.gpsimd.indirect_dma_start(
                    out=g2[:], out_offset=None, in_=y_recv[:],
                    in_offset=bass.IndirectOffsetOnAxis(
                        ap=pos_i[:, 2 * tm + 1:2 * tm + 2], axis=0))
                yc = cmb.tile([128, D], F32, tag="yc")
                t2 = cmb.tile([128, D], F32, tag="t2")
                nc.vector.tensor_scalar_mul(yc[:], g1[:], wsv[:, 2 * tm:2 * tm + 1])
                nc.vector.tensor_scalar_mul(t2[:], g2[:],
                                            wsv[:, 2 * tm + 1:2 * tm + 2])
                nc.vector.tensor_tensor(yc[:], yc[:], t2[:], OP.add)
                for k in range(DK):
                    ptd = ptr.tile([128, 128], F32, tag="ptr")
                    nc.tensor.transpose(ptd[:], yc[:, k * 128:(k + 1) * 128],
                                        ident[:])
                    mo = sbt.tile([128, 128], F32, tag="mo128")
                    nc.vector.tensor_tensor(mo[:], ptd[:],
                                            dsT[:, k, tm * 128:(tm + 1) * 128],
                                            OP.add)
                    nc.vector.tensor_scalar_mul(mo[:], mo[:], 0.5)
                    nc.vector.tensor_tensor(xln2[:, k, tm * 128:(tm + 1) * 128],
                                            mo[:], xT[:, k, tm * 128:(tm + 1) * 128],
                                            OP.add)
        layer_norm_(xT, xln2, lb["ln2w"], lb["ln2b"], 1e-5, bdst=xB)
        dsT_pool.__exit__(None, None, None)

    dbg("xfinal", xT[:])
    # ---------------- final rms + allgather + lm_head
    rmsw = cst.tile([128, DK], F32, name="rmsw_sb")
    nc.sync.dma_start(rmsw[:], P["rmsw"][:])
    LMDT0 = BF16 if c.get("lm_bf16", True) else F32R
    xf_in = drp.tile([128, DK, TC], LMDT0, name="xfin")
    xf_all = drp.tile([NC, 128, DK, TC], LMDT0, name="xfall", addr_space="Shared")
    xr = xlp.tile([128, DK, TC], F32R, tag="xln")
    layer_norm_(xr, xT, rmsw, None, 1e-6, skip_mean=True)
    LMDT = BF16 if c.get("lm_bf16", True) else F32R
    xrb = xlp.tile([128, DK, TC], LMDT, tag="xrb")
    for k in range(DK):
        nc.vector.tensor_copy(xrb[:, k, :], xr[:, k, :])
    nc.sync.dma_start(xf_in[:], xrb[:])
    nc.gpsimd.collective_compute(
        "AllGather", OP.bypass, replica_groups=GRP_ALL,
        ins=[xf_in[:]], outs=[xf_all[:]])
    with (
        tc.tile_pool(name="lph", bufs=2) as lph,
        tc.tile_pool(name="lphw", bufs=8) as lphw,
    ):
        for n in range(NC):
            xfn = lph.tile([128, DK, TC], LMDT0, tag="xan")
            nc.sync.dma_start(xfn[:], xf_all[n])
            for m in range(VCK):
                wt = lphw.tile([128, DK, 128], LMDT0, tag="wt")
                nc.sync.dma_start(wt[:], P["embT"][m])
                ps = psm.tile([128, TC], F32, tag="psmm")
                for k in range(DK):
                    nc.tensor.matmul(ps[:], wt[:, k, :], xfn[:, k, :],
                                     start=(k == 0), stop=(k == DK - 1))
                lo = sbt.tile([128, TC], F32, tag="lo")
                nc.vector.tensor_copy(lo[:], ps[:])
                rows = min(128, VC - m * 128)
                nc.sync.dma_start(
                    OUT[m * 128:m * 128 + rows, n * TC:(n + 1) * TC], lo[:rows, :])

    es.close()


# ---------------------------------------------------------------- runner

def run_model(inputs, cfg, nc=None):
    c = derived(cfg)
    in_maps = prep_in_maps(inputs, cfg)
    if nc is None:
        nc = build_nc(cfg)
    res = run_bass_kernel_spmd(nc, in_maps, core_ids=list(range(c["NC"])))
    return assemble_logits(res.results, cfg), nc


# ---------------------------------------------------------------- entry point

_NC_CACHE = None


def kernel(**inputs):
    """Full-model forward on 8 trn2 cores. inputs as in reference.setup_inputs()."""
    global _NC_CACHE
    import numpy as _np
    inputs = {k: _np.asarray(v) for k, v in inputs.items()}
    if _NC_CACHE is None:
        _NC_CACHE = build_nc(FULL_CFG)
    in_maps = prep_in_maps(inputs, FULL_CFG)
    res = run_bass_kernel_spmd(_NC_CACHE, in_maps,
                               core_ids=list(range(FULL_CFG["NC"])))
    return assemble_logits(res.results, FULL_CFG)

